# revision 30
# baseline (speedup 1.0000x reference)
"""Multi-head attention (AttnProcessor2_0) on 8 TRN2 NeuronCores.

Problem: B=2, S=4096, C=640, H=10, Dh=64.
  q/k/v = hs @ W{q,k,v}.T ; per-head scores = q k^T / 8 ; softmax ;
  out = probs v ; y = out @ Wo.T + b_out + hs

Sharding (no collectives): core c -> batch b=c//4, query block g=c%4
(1024 queries).  Each core recomputes full K/V for its batch (head-dim
on partitions), computes its own S/4 x S attention block, output
projection, bias+residual.  Host passes hidden states TRANSPOSED and
ROLLED by the query offset so the same SPMD program works on every
core (softmax+PV are permutation-invariant along the key axis).

v3 layout (vs the 514us baseline):
  * PV fp8 DoubleRow: probs written by ScalarE exp directly as fp8
    (e5m2 -- e4m3 stores cost ScalarE ~20% extra, e5m2 runs at bf16
    rate), v projected to fp8e4 with key-chunk PAIRS packed at stride
    656; each DR matmul contracts 256 keys (moving free dim 1024 >=
    256 so the ~1.5x DR win applies).  Denominators still fall out as
    PSUM row 64 via the ones column.  PV: 327,680 -> 163,840 cycles.
  * Q/K/V projections in fp8 DoubleRow: hs and Wq/Wk/Wv uploaded as
    fp8e4 in chunk-major [128, chunk, n] layout so DR pairs feature
    chunks along the free axis; 640-contraction = 2 DR + 1 plain
    matmul (5 -> 3 instructions per tile).
  * O proj de-padded: attention outputs are packed per head-PAIR
    [128, SQ], so the 640-contraction runs dense (25,600 cycles).
  * Normalization off the pv critical path: numerator+denominator rows
    are copied out of PSUM immediately (releasing the bank), the bf16
    rank-1 reciprocal broadcast goes to a transient pp bank, and the
    pending broadcasts pop at gi 5/7 of the NEXT pair so the ~3.4us
    DVE reciprocal never stalls the strict-FIFO PE queue.
  * QK stays zero-padded full-K (K=64 matmuls or 64-row tiles drop
    HAM's utilization accounting below its limit and the PE is
    clock-gated to 1.2 GHz).  PSUM: 4 banks scores (A/B ping-pong) +
    2 pv + 2 background-projection banks.
"""

import sys

if "/opt/trn_rl_repo" not in sys.path:
    sys.path.insert(0, "/opt/trn_rl_repo")

from contextlib import ExitStack

import ml_dtypes
import numpy as np

import concourse.bass as bass
import concourse.tile as tile
from concourse import mybir
from concourse.bass import ts

BF16 = mybir.dt.bfloat16
F32 = mybir.dt.float32
FP8 = mybir.dt.float8e4  # v tiles (e4m3: 3-bit mantissa)
PT8 = mybir.dt.float8e5  # probs from exp (e5m2)

B, S, C = 2, 4096, 640
H, DH = 10, 64
NCORES = 8
GROUP = 4  # cores per batch element
SQ = S // GROUP  # 1024 queries per core
SCALE = 0.125  # 1/sqrt(64)
CCH = C // 128  # 5 feature chunks (2 heads each)
NJT = S // 512  # 8 key tiles for K proj
NJC = S // 128  # 32 key chunks for attention
NJP = NJC // 2  # 16 key chunk PAIRS (DoubleRow k-tiles)
NIT = SQ // 512  # 2 query tiles
VST = DH + 1  # 65: per-head stride in v tiles (ones col at f=64)
VKT = 656  # per-ktile stride in v pair tiles (10*65=650 padded to %16==0)

# Schraudolph exp-as-int8 on DVE: writing round(A*score + B) as int8 and
# bitcasting the bytes as e5m2 IS exp(score*SCALE) to within ~3% (the
# mantissa-linear 2^f approximation) -- same order as the e5m2 grid the
# ScalarE path rounds onto anyway.  Offloading a fixed subset of exp
# groups to DVE relieves the ScalarE bottleneck (the only engine with a
# real exp) and keeps the PE the busiest engine, which also keeps HAM's
# activity monitor from clock-gating it.  B folds the e5m2 bias (15) and
# the half-max 2^f correction (-0.0436).
EXP_A = 4.0 * 1.4426950408889634 * SCALE
EXP_B = 4.0 * (15.0 - 0.0436)
# measured: DVE int8 stores run at HALF rate (2.1us per 1024-col call vs
# ScalarE's 1.11us) -- the offload only pays if DVE is otherwise idle,
# which it is not.  Disabled; ScalarE keeps all exp groups.
DVE_EXP_GROUPS = ()


def build_nc() -> bass.Bass:
    nc = bass.Bass()
    # hs and the q/k/v weights come in as fp8e4 in chunk-major layout
    # [128, chunk, n] so DoubleRow can pair feature chunks along the free
    # axis (the two k-tiles of a DR matmul must share partitions).
    hs2 = nc.declare_dram_parameter("hs2", [128, CCH * S], FP8, isOutput=False)
    res = nc.declare_dram_parameter("res", [C, SQ], F32, isOutput=False)
    wq2 = nc.declare_dram_parameter("wq2", [128, CCH * C], FP8, isOutput=False)
    wk2 = nc.declare_dram_parameter("wk2", [128, CCH * C], FP8, isOutput=False)
    wv2 = nc.declare_dram_parameter("wv2", [128, CCH * C], FP8, isOutput=False)
    woT = nc.declare_dram_parameter("woT", [C, C], BF16, isOutput=False)
    out = nc.declare_dram_parameter("out", [C, SQ], F32, isOutput=True)

    with ExitStack() as ctx:
        tc = ctx.enter_context(tile.TileContext(nc))
        # outer pool: tensors whose lifetime spans projections AND attention
        sb = ctx.enter_context(tc.tile_pool(name="sb", bufs=1))

        kT_sb = [sb.tile([128, S], BF16, tag=f"kT{i}", name=f"kT{i}") for i in range(CCH)]
        # per-head q, zero-padded to full 128-row contraction: partial-K
        # (K=64) matmuls drop HAM's utilization accounting below its 0.5
        # limit and clock the PE down to 1.2 GHz -- padding the contraction
        # with zero rows is exact and keeps the array at the warm rate.
        qTz_sb = [
            [sb.tile([128, SQ], BF16, tag=f"qz{i}_{p}", name=f"qz{i}_{p}")
             for p in range(2)]
            for i in range(CCH)
        ]
        # v: fp8, key-chunk pairs side by side (ktile stride 656 for DR)
        v_sb = [sb.tile([128, 2 * VKT], FP8, tag=f"v{j}", name=f"v{j}") for j in range(NJP)]
        # attention outputs packed per head pair (no zero padding)
        attn_sb = [sb.tile([128, SQ], BF16, tag=f"attn{h}", name=f"attn{h}")
                   for h in range(CCH)]
        ones_sb = sb.tile([128, DH], BF16, tag="ones", name="ones")
        nc.vector.memset(ones_sb[:], 1.0)

        # ---------------- load + first projections ----------------
        load = ctx.enter_context(tc.tile_pool(name="load", bufs=1))
        hs_sb = load.tile([128, CCH * S], FP8, tag="hs2", name="hs2")
        wk_sb = load.tile([128, CCH * C], FP8, tag="wk2", name="wk2")
        wq_sb = load.tile([128, CCH * C], FP8, tag="wq2", name="wq2")
        wv_sb = load.tile([128, CCH * C], FP8, tag="wv2", name="wv2")
        nc.sync.dma_start(wk_sb[:], wk2[:, :])
        for cc in range(CCH):
            nc.sync.dma_start(
                hs_sb[:, cc * S : cc * S + SQ], hs2[:, cc * S : cc * S + SQ]
            )
        nc.sync.dma_start(wq_sb[:], wq2[:, :])
        nc.sync.dma_start(wv_sb[:], wv2[:, :])
        h3 = hs_sb[:].rearrange("p (c s) -> p c s", c=CCH)
        k3 = wk_sb[:].rearrange("p (c f) -> p c f", c=CCH)
        q3 = wq_sb[:].rearrange("p (c f) -> p c f", c=CCH)
        v3w = wv_sb[:].rearrange("p (c f) -> p c f", c=CCH)

        def emit_hsT_tail():
            # deferred until after the first exp so ScalarE's conservative
            # vector-clock waits don't cover this 2MB of DMA
            for blk in range(SQ, S, SQ):
                for cc in range(CCH):
                    nc.sync.dma_start(
                        hs_sb[:, cc * S + blk : cc * S + blk + SQ],
                        hs2[:, cc * S + blk : cc * S + blk + SQ],
                    )

        def _proj_mm(ps, w3, wsl, hsl, dn=512):
            # contraction over 640 = 2 fp8 DoubleRow k-tile pairs + 1 plain
            nc.tensor.matmul(
                ps[:, 0:dn], w3[:, 0:2, wsl], h3[:, 0:2, hsl],
                start=True, stop=False,
                perf_mode=mybir.MatmulPerfMode.DoubleRow,
            )
            nc.tensor.matmul(
                ps[:, 0:dn], w3[:, 2:4, wsl], h3[:, 2:4, hsl],
                start=False, stop=False,
                perf_mode=mybir.MatmulPerfMode.DoubleRow,
            )
            nc.tensor.matmul(
                ps[:, 0:dn], w3[:, 4, wsl], h3[:, 4, hsl],
                start=False, stop=True,
            )

        def emit_kproj(dc, jt, pool):
            ps = pool.tile([128, 512], F32, tag="pp", name="pp", bufs=2)
            _proj_mm(ps, k3, ts(dc, 128), ts(jt, 512))
            nc.vector.tensor_copy(kT_sb[dc][:, ts(jt, 512)], ps[:])

        def emit_qproj(dc, it, pool):
            ps = pool.tile([128, 512], F32, tag="pp", name="pp", bufs=2)
            _proj_mm(ps, q3, ts(dc, 128), ts(it, 512))
            nc.vector.tensor_copy(qTz_sb[dc][0][0:DH, ts(it, 512)], ps[0:DH, :])
            nc.vector.tensor_copy(qTz_sb[dc][1][DH:128, ts(it, 512)], ps[DH:128, :])

        def emit_vproj(jc, pool):
            jp, half = divmod(jc, 2)
            vt = v_sb[jp]
            v3 = vt[:, half * VKT : half * VKT + H * VST].rearrange(
                "p (h x) -> p h x", x=VST
            )
            nc.vector.memset(v3[:, :, DH : DH + 1], 1.0)
            for d0, dn in ((0, 512), (512, 128)):
                ps = pool.tile([128, 512], F32, tag="pp", name="pp", bufs=2)
                # stationary = hs chunk pairs (M=128 keys), moving = wv
                nc.tensor.matmul(
                    ps[:, 0:dn], h3[:, 0:2, ts(jc, 128)],
                    v3w[:, 0:2, d0 : d0 + dn],
                    start=True, stop=False,
                    perf_mode=mybir.MatmulPerfMode.DoubleRow,
                )
                nc.tensor.matmul(
                    ps[:, 0:dn], h3[:, 2:4, ts(jc, 128)],
                    v3w[:, 2:4, d0 : d0 + dn],
                    start=False, stop=False,
                    perf_mode=mybir.MatmulPerfMode.DoubleRow,
                )
                nc.tensor.matmul(
                    ps[:, 0:dn], h3[:, 4, ts(jc, 128)],
                    v3w[:, 4, d0 : d0 + dn],
                    start=False, stop=True,
                )
                nc.vector.tensor_copy(
                    v3[:, d0 // DH : (d0 + dn) // DH, 0:DH],
                    ps[:, 0:dn].rearrange("p (h x) -> p h x", x=DH),
                )

        with tc.tile_pool(name="pp0", bufs=2, space="PSUM") as pp0:
            for dc in range(CCH):
                nc.vector.memset(qTz_sb[dc][0][DH:128, :], 0.0)
                nc.vector.memset(qTz_sb[dc][1][0:DH, :], 0.0)
            # minimal prologue: the first exp only needs kT chunk jt0 and
            # it0's q; everything else rides in the background list
            emit_kproj(0, 0, pp0)
            emit_qproj(0, 0, pp0)

        # ---------------- attention phase ----------------
        with tc.tile_pool(name="ap", bufs=1, space="PSUM") as ap, \
             tc.tile_pool(name="pt", bufs=6) as pt_pool, \
             tc.tile_pool(name="ob", bufs=3) as ob, \
             tc.tile_pool(name="scratch", bufs=4) as scratch:

            def norm_dve(hp, par, pv, p_isl):
                # copy numerator + denominator row out of PSUM (releases the
                # pv bank), then approx-reciprocal the denominators on DVE
                # (2.8x faster than InstReciprocal, ~2 ULP).
                raw = scratch.tile([DH, 512], BF16, tag="raw", name="raw")
                nc.vector.tensor_copy(raw[:], pv[0:DH, :])
                den = scratch.tile([DH + 1, 512], F32, tag="den", name="den")
                nc.vector.tensor_copy(den[DH : DH + 1, :], pv[DH : DH + 1, :])
                rec = scratch.tile([DH + 1, 512], BF16, tag="rec", name="rec")
                with nc.allow_low_precision(reason="softmax recip bf16"):
                    nc.vector.reciprocal(rec[DH : DH + 1, :], den[DH : DH + 1, :])
                return (hp, par, p_isl, raw, rec)

            def norm_mul(state, anchor=None):
                # rank-1 PE outer product broadcasts the reciprocal row
                # across partitions into a transient pp bank; DVE multiplies.
                # The tile scheduler orders by data deps, which would place
                # this right after the pair's last PV where it blocks the
                # strict-FIFO PE for the full ~3.4us reciprocal; pin it
                # behind a mid-next-pair QK instead.
                hp, par, p_isl, raw, rec = state
                rb = ap.tile([128, 512], F32, tag="pp", bufs=2, name="pp")
                r_mm = nc.tensor.matmul(
                    rb[0:DH, :],
                    ones_sb[DH : DH + 1, :],
                    rec[DH : DH + 1, :],
                    start=True,
                    stop=True,
                )
                if anchor is not None:
                    tile.add_dep_helper(
                        r_mm.ins, anchor.ins, sync=False,
                        reason="norm rank1 after anchor QK (hide recip latency)",
                    )
                nc.vector.tensor_mul(
                    attn_sb[hp][par * DH : (par + 1) * DH, p_isl],
                    raw[:],
                    rb[0:DH, :],
                )

            def emit_oproj(ec, it):
                wos = []
                for hp in range(CCH):
                    wt = ob.tile([128, 128], BF16, tag="woec", name="woec",
                                 bufs=25)
                    nc.sync.dma_start(wt[:], woT[ts(hp, 128), ts(ec, 128)])
                    wos.append(wt)
                ps = ap.tile([128, 512], F32, tag="pp", name="pp", bufs=2)
                for hp in range(CCH):
                    nc.tensor.matmul(
                        ps[:],
                        wos[hp][:],
                        attn_sb[hp][:, ts(it, 512)],
                        start=(hp == 0),
                        stop=(hp == CCH - 1),
                    )
                rt = ob.tile([128, 512], F32, tag="rt", name="rt", bufs=5)
                nc.sync.dma_start(rt[:], res[ts(ec, 128), ts(it, 512)])
                ot = ob.tile([128, 512], F32, tag="ot", name="ot", bufs=5)
                nc.vector.tensor_add(ot[:], ps[:], rt[:])
                nc.sync.dma_start(out[ts(ec, 128), ts(it, 512)], ot[:])

            pending = []
            bg = [
                (lambda jt=jt: emit_kproj(0, jt, ap)) for jt in range(1, NJT)
            ]
            bg.append(lambda: emit_qproj(0, 1, ap))
            for it in range(NIT):
                isl = ts(it, 512)
                for hp in range(CCH):
                    if it == 0 and hp + 1 < CCH:
                        # prefetch next pair's K/Q chunks in the background
                        bg.extend(
                            (lambda jt=jt, dc=hp + 1: emit_kproj(dc, jt, ap))
                            for jt in range(NJT)
                        )
                        bg.extend(
                            (lambda q_it=q_it, dc=hp + 1: emit_qproj(dc, q_it, ap))
                            for q_it in range(NIT)
                        )
                    if it == 1 and hp >= 1:
                        # spread the it0 output projections across the it1
                        # pairs: it1 has no projection prefetch, and a starved
                        # PE micro-idles every slot, making HAM oscillate
                        ecs = [hp - 1] if hp < 4 else [3, 4]
                        bg.extend(
                            (lambda ec=ec: emit_oproj(ec, 0)) for ec in ecs
                        )
                    pvA = ap.tile([DH + 1, 512], F32, tag="pv", bufs=2,
                                  name="pv")
                    pvB = ap.tile([DH + 1, 512], F32, tag="pv", bufs=2,
                                  name="pv")
                    for gi in range(NJP):
                        scA = ap.tile([128, 1024], F32, tag="scA", bufs=1,
                                      name="scA")
                        scB = ap.tile([128, 1024], F32, tag="scB", bufs=1,
                                      name="scB")
                        # full-K QK (zero-padded moving q keeps HAM warm)
                        last_qk = None
                        for k, jc in enumerate((2 * gi, 2 * gi + 1)):
                            nc.tensor.matmul(
                                scA[:, ts(k, 512)],
                                kT_sb[hp][:, ts(jc, 128)],
                                qTz_sb[hp][0][:, isl],
                                start=True,
                                stop=True,
                            )
                            last_qk = nc.tensor.matmul(
                                scB[:, ts(k, 512)],
                                kT_sb[hp][:, ts(jc, 128)],
                                qTz_sb[hp][1][:, isl],
                                start=True,
                                stop=True,
                            )
                        ptA = pt_pool.tile([128, 1024], PT8, tag="pt",
                                           name="pt")
                        ptB = pt_pool.tile([128, 1024], PT8, tag="pt",
                                           name="pt")
                        if gi in DVE_EXP_GROUPS:
                            for pt, sc in ((ptA, scA), (ptB, scB)):
                                nc.vector.tensor_scalar(
                                    pt[:].bitcast(mybir.dt.int8),
                                    sc[:],
                                    EXP_A,
                                    EXP_B,
                                    mybir.AluOpType.mult,
                                    mybir.AluOpType.add,
                                )
                        else:
                            for pt, sc in ((ptA, scA), (ptB, scB)):
                                nc.scalar.activation(
                                    pt[:], sc[:],
                                    mybir.ActivationFunctionType.Exp,
                                    bias=0.0, scale=SCALE,
                                )
                        # pop the previous pair's normalizations late enough
                        # that the ~3.4us DVE reciprocal has finished: the
                        # rank-1 broadcast sits in the strict-FIFO PE queue,
                        # and popping it early stalls every QK behind it
                        if pending and gi == 4:
                            norm_mul(pending.pop(0), last_qk)
                        if pending and gi == 6:
                            norm_mul(pending.pop(0), last_qk)
                        if it == 0 and hp == 0:
                            if gi == 0:
                                emit_hsT_tail()
                            # V projection rides inside the first pair's
                            # window, each chunk pair just ahead of its PV
                            if bg:
                                bg.pop(0)()
                            emit_vproj(2 * gi, ap)
                            emit_vproj(2 * gi + 1, ap)
                        elif bg:
                            bg.pop(0)()
                        # PV: fp8 DoubleRow over the key-chunk pair
                        vv = v_sb[gi][:].rearrange("p (t z) -> p t z", t=2)
                        for par, pv, pt in ((0, pvA, ptA), (1, pvB, ptB)):
                            h = 2 * hp + par
                            nc.tensor.matmul(
                                pv[:],
                                vv[:, :, h * VST : h * VST + VST],
                                pt[:].rearrange("p (t n) -> p t n", t=2),
                                start=(gi == 0),
                                stop=(gi == NJP - 1),
                                perf_mode=mybir.MatmulPerfMode.DoubleRow,
                            )
                    pending.append(norm_dve(hp, 0, pvA, isl))
                    pending.append(norm_dve(hp, 1, pvB, isl))
                    # barrier: next pair's kT/qT must be fully emitted
                    # before its first QK reads them
                    while bg:
                        bg.pop(0)()
            for st in pending:
                norm_mul(st)
            for ec in range(CCH):
                emit_oproj(ec, 1)

    _spill_matmul_waits(nc)
    return nc


# walrus embedded-sync-wait capacity per BIR opcode.  Matmult holds a
# single wait; excess waits hoist onto the paired Ldweights (in-order
# issue on PE makes that equivalent).  Other compute ops spill onto
# EventSemaphore carrier instructions inserted just before them on the
# same engine.  DMACopy / Drain / EventSemaphore handle many waits
# natively (bacc emits such itself) and are left alone.
_WAIT_CAPS = {
    "InstMatmult": 1,
    "InstLdweights": 1,
    "InstActivation": 1,
    "InstReciprocal": 1,
    "InstTensorTensor": 1,
    "InstTensorCopy": 1,
    "InstTensorScalarPtr": 1,
    "InstTensorReduce": 1,
    "InstMemset": 1,
    "InstDMACopy": 1,
    "InstDrain": 1,
    "InstCustomDveAnt": 1,
}
_ES_CAP = 2  # waits per EventSemaphore carrier (walrus: <=2 waits, <=1 update)


def _spill_matmul_waits(nc: bass.Bass) -> None:
    spill_id = [0]

    def carriers(excess, engine):
        out = []
        for i in range(0, len(excess), _ES_CAP):
            es = mybir.InstEventSemaphore(
                name=f"wait-spill-{spill_id[0]}", ins=[], outs=[]
            )
            spill_id[0] += 1
            es.engine = engine
            es.sync_info = mybir.SyncInfo(
                on_wait=excess[i : i + _ES_CAP], on_update=[]
            )
            out.append(es)
        return out

    for f in nc.m.functions:
        for blk in f.blocks:
            insts = blk.instructions
            i = 0
            while i < len(insts):
                inst = insts[i]
                tn = type(inst).__name__
                cap = _WAIT_CAPS.get(tn)
                si = inst.sync_info
                if cap is None or si is None or len(si.on_wait) <= cap:
                    i += 1
                    continue
                w = list(si.on_wait)
                if tn == "InstMatmult" and cap == 1:
                    # Keep the latest-satisfied dependency (the ACT-produced
                    # operand, e.g. probs from exp) embedded on the matmul and
                    # hoist early ones onto the Ldweights: a wait on the LDW
                    # blocks its background prefetch and serializes ~50ns of
                    # weight-load into every PV matmul.
                    acts = [x for x in w if "Activation" in (x.ant_name or "")]
                    if acts:
                        keep = [acts[-1]]
                        excess = [x for x in w if x is not acts[-1]]
                    else:
                        keep, excess = w[-cap:], w[:-cap]
                else:
                    keep, excess = w[-cap:], w[:-cap]
                prev = insts[i - 1] if i > 0 else None
                if (
                    tn == "InstMatmult"
                    and prev is not None
                    and type(prev).__name__ == "InstLdweights"
                    and len(((prev.sync_info and prev.sync_info.on_wait) or []))
                    + len(excess) <= 1
                ):
                    psi = prev.sync_info
                    pw = list(psi.on_wait) if psi is not None else []
                    pu = list(psi.on_update) if psi is not None else []
                    prev.sync_info = mybir.SyncInfo(on_wait=pw + excess, on_update=pu)
                else:
                    new = carriers(excess, inst.engine)
                    insts[i:i] = new
                    i += len(new)
                inst.sync_info = mybir.SyncInfo(
                    on_wait=keep, on_update=list(si.on_update)
                )
                i += 1


_CACHED_NC = None


def get_nc() -> bass.Bass:
    global _CACHED_NC
    if _CACHED_NC is None:
        _CACHED_NC = build_nc()
    return _CACHED_NC


def _chunk_major(mT, n, f8):
    # [C, n] -> [128, CCH*n] fp8: row p, col cc*n+j = mT[128*cc + p, j]
    return np.ascontiguousarray(
        mT.reshape(CCH, 128, n).transpose(1, 0, 2).reshape(128, CCH * n)
    ).astype(f8)


def make_in_maps(hidden_states, Wq, Wk, Wv, Wo, b_out):
    hs = np.asarray(hidden_states, dtype=np.float32)
    bf = ml_dtypes.bfloat16
    f8 = mybir.dt.np(FP8)
    wq2 = _chunk_major(np.asarray(Wq, np.float32).T, C, f8)
    wk2 = _chunk_major(np.asarray(Wk, np.float32).T, C, f8)
    wv2 = _chunk_major(np.asarray(Wv, np.float32).T, C, f8)
    woT = np.ascontiguousarray(np.asarray(Wo, np.float32).T).astype(bf)
    bias = np.asarray(b_out, np.float32).reshape(C, 1)
    in_maps = []
    for c in range(NCORES):
        b, g = divmod(c, GROUP)
        i0 = g * SQ
        hsTb = hs[b].T  # [C, S]
        in_maps.append(
            {
                "hs2": _chunk_major(np.roll(hsTb, -i0, axis=1), S, f8),
                "res": np.ascontiguousarray(hsTb[:, i0 : i0 + SQ]) + bias,
                "wq2": wq2,
                "wk2": wk2,
                "wv2": wv2,
                "woT": woT,
            }
        )
    return in_maps


def assemble(results) -> np.ndarray:
    y = np.empty((B, S, C), np.float32)
    for c in range(NCORES):
        b, g = divmod(c, GROUP)
        i0 = g * SQ
        y[b, i0 : i0 + SQ, :] = np.asarray(results[c]["out"], np.float32).T
    return y


def kernel(**inputs) -> np.ndarray:
    from concourse.bass_utils import run_bass_kernel_spmd

    nc = get_nc()
    in_maps = make_in_maps(**inputs)
    res = run_bass_kernel_spmd(nc, in_maps, list(range(NCORES)))
    return assemble(res.results)


if __name__ == "__main__":
    import reference

    inputs = {k: np.asarray(v) for k, v in reference.setup_inputs().items()}
    got = kernel(**inputs)
    want = np.asarray(reference.reference(**inputs))
    err = np.linalg.norm(got - want) / np.linalg.norm(want)
    print("Relative error:", err)


# revision 33
# speedup vs baseline: 1.0040x; 1.0040x over previous
"""Multi-head attention (AttnProcessor2_0) on 8 TRN2 NeuronCores.

Problem: B=2, S=4096, C=640, H=10, Dh=64.
  q/k/v = hs @ W{q,k,v}.T ; per-head scores = q k^T / 8 ; softmax ;
  out = probs v ; y = out @ Wo.T + b_out + hs

Sharding (no collectives): core c -> batch b=c//4, query block g=c%4
(1024 queries).  Each core recomputes full K/V for its batch (head-dim
on partitions), computes its own S/4 x S attention block, output
projection, bias+residual.  Host passes hidden states TRANSPOSED and
ROLLED by the query offset so the same SPMD program works on every
core (softmax+PV are permutation-invariant along the key axis).

v3 layout (vs the 514us baseline):
  * PV fp8 DoubleRow: probs written by ScalarE exp directly as fp8
    (e5m2 -- e4m3 stores cost ScalarE ~20% extra, e5m2 runs at bf16
    rate), v projected to fp8e4 with key-chunk PAIRS packed at stride
    656; each DR matmul contracts 256 keys (moving free dim 1024 >=
    256 so the ~1.5x DR win applies).  Denominators still fall out as
    PSUM row 64 via the ones column.  PV: 327,680 -> 163,840 cycles.
  * Q/K/V projections in fp8 DoubleRow: hs and Wq/Wk/Wv uploaded as
    fp8e4 in chunk-major [128, chunk, n] layout so DR pairs feature
    chunks along the free axis; 640-contraction = 2 DR + 1 plain
    matmul (5 -> 3 instructions per tile).
  * O proj de-padded: attention outputs are packed per head-PAIR
    [128, SQ], so the 640-contraction runs dense (25,600 cycles).
  * Normalization off the pv critical path: numerator+denominator rows
    are copied out of PSUM immediately (releasing the bank), the bf16
    rank-1 reciprocal broadcast goes to a transient pp bank, and the
    pending broadcasts pop at gi 5/7 of the NEXT pair so the ~3.4us
    DVE reciprocal never stalls the strict-FIFO PE queue.
  * QK stays zero-padded full-K (K=64 matmuls or 64-row tiles drop
    HAM's utilization accounting below its limit and the PE is
    clock-gated to 1.2 GHz).  PSUM: 4 banks scores (A/B ping-pong) +
    2 pv + 2 background-projection banks.
"""

import sys

if "/opt/trn_rl_repo" not in sys.path:
    sys.path.insert(0, "/opt/trn_rl_repo")

from contextlib import ExitStack

import ml_dtypes
import numpy as np

import concourse.bass as bass
import concourse.tile as tile
from concourse import mybir
from concourse.bass import ts

BF16 = mybir.dt.bfloat16
F32 = mybir.dt.float32
FP8 = mybir.dt.float8e4  # v tiles (e4m3: 3-bit mantissa)
PT8 = mybir.dt.float8e5  # probs from exp (e5m2)

B, S, C = 2, 4096, 640
H, DH = 10, 64
NCORES = 8
GROUP = 4  # cores per batch element
SQ = S // GROUP  # 1024 queries per core
SCALE = 0.125  # 1/sqrt(64)
CCH = C // 128  # 5 feature chunks (2 heads each)
NJT = S // 512  # 8 key tiles for K proj
NJC = S // 128  # 32 key chunks for attention
NJP = NJC // 2  # 16 key chunk PAIRS (DoubleRow k-tiles)
NIT = SQ // 512  # 2 query tiles
VST = DH + 1  # 65: per-head stride in v tiles (ones col at f=64)
VKT = 656  # per-ktile stride in v pair tiles (10*65=650 padded to %16==0)

# Schraudolph exp-as-int8 on DVE: writing round(A*score + B) as int8 and
# bitcasting the bytes as e5m2 IS exp(score*SCALE) to within ~3% (the
# mantissa-linear 2^f approximation) -- same order as the e5m2 grid the
# ScalarE path rounds onto anyway.  Offloading a fixed subset of exp
# groups to DVE relieves the ScalarE bottleneck (the only engine with a
# real exp) and keeps the PE the busiest engine, which also keeps HAM's
# activity monitor from clock-gating it.  B folds the e5m2 bias (15) and
# the half-max 2^f correction (-0.0436).
EXP_A = 4.0 * 1.4426950408889634 * SCALE
EXP_B = 4.0 * (15.0 - 0.0436)
# measured: DVE int8 stores run at HALF rate (2.1us per 1024-col call vs
# ScalarE's 1.11us) -- the offload only pays if DVE is otherwise idle,
# which it is not.  Disabled; ScalarE keeps all exp groups.
DVE_EXP_GROUPS = ()


def build_nc() -> bass.Bass:
    nc = bass.Bass()
    # hs and the q/k/v weights come in as fp8e4 in chunk-major layout
    # [128, chunk, n] so DoubleRow can pair feature chunks along the free
    # axis (the two k-tiles of a DR matmul must share partitions).
    hs2 = nc.declare_dram_parameter("hs2", [128, CCH * S], FP8, isOutput=False)
    res = nc.declare_dram_parameter("res", [C, SQ], F32, isOutput=False)
    wq2 = nc.declare_dram_parameter("wq2", [128, CCH * C], FP8, isOutput=False)
    wk2 = nc.declare_dram_parameter("wk2", [128, CCH * C], FP8, isOutput=False)
    wv2 = nc.declare_dram_parameter("wv2", [128, CCH * C], FP8, isOutput=False)
    woT = nc.declare_dram_parameter("woT", [C, C], BF16, isOutput=False)
    out = nc.declare_dram_parameter("out", [C, SQ], F32, isOutput=True)

    with ExitStack() as ctx:
        tc = ctx.enter_context(tile.TileContext(nc))
        # outer pool: tensors whose lifetime spans projections AND attention
        sb = ctx.enter_context(tc.tile_pool(name="sb", bufs=1))

        kT_sb = [sb.tile([128, S], BF16, tag=f"kT{i}", name=f"kT{i}") for i in range(CCH)]
        # per-head q, zero-padded to full 128-row contraction: partial-K
        # (K=64) matmuls drop HAM's utilization accounting below its 0.5
        # limit and clock the PE down to 1.2 GHz -- padding the contraction
        # with zero rows is exact and keeps the array at the warm rate.
        qTz_sb = [
            [sb.tile([128, SQ], BF16, tag=f"qz{i}_{p}", name=f"qz{i}_{p}")
             for p in range(2)]
            for i in range(CCH)
        ]
        # v: fp8, key-chunk pairs side by side (ktile stride 656 for DR)
        v_sb = [sb.tile([128, 2 * VKT], FP8, tag=f"v{j}", name=f"v{j}") for j in range(NJP)]
        # attention outputs packed per head pair (no zero padding)
        attn_sb = [sb.tile([128, SQ], BF16, tag=f"attn{h}", name=f"attn{h}")
                   for h in range(CCH)]
        ones_sb = sb.tile([128, DH], BF16, tag="ones", name="ones")
        nc.vector.memset(ones_sb[:], 1.0)

        # ---------------- load + first projections ----------------
        load = ctx.enter_context(tc.tile_pool(name="load", bufs=1))
        hs_sb = load.tile([128, CCH * S], FP8, tag="hs2", name="hs2")
        wk_sb = load.tile([128, CCH * C], FP8, tag="wk2", name="wk2")
        wq_sb = load.tile([128, CCH * C], FP8, tag="wq2", name="wq2")
        wv_sb = load.tile([128, CCH * C], FP8, tag="wv2", name="wv2")
        nc.sync.dma_start(wk_sb[:], wk2[:, :])
        for cc in range(CCH):
            nc.sync.dma_start(
                hs_sb[:, cc * S : cc * S + SQ], hs2[:, cc * S : cc * S + SQ]
            )
        nc.sync.dma_start(wq_sb[:], wq2[:, :])
        nc.sync.dma_start(wv_sb[:], wv2[:, :])
        h3 = hs_sb[:].rearrange("p (c s) -> p c s", c=CCH)
        k3 = wk_sb[:].rearrange("p (c f) -> p c f", c=CCH)
        q3 = wq_sb[:].rearrange("p (c f) -> p c f", c=CCH)
        v3w = wv_sb[:].rearrange("p (c f) -> p c f", c=CCH)

        def emit_hsT_tail():
            # deferred until after the first exp so ScalarE's conservative
            # vector-clock waits don't cover this 2MB of DMA
            for blk in range(SQ, S, SQ):
                for cc in range(CCH):
                    nc.sync.dma_start(
                        hs_sb[:, cc * S + blk : cc * S + blk + SQ],
                        hs2[:, cc * S + blk : cc * S + blk + SQ],
                    )

        def _proj_mm(ps, w3, wsl, hsl, dn=512):
            # contraction over 640 = 2 fp8 DoubleRow k-tile pairs + 1 plain
            nc.tensor.matmul(
                ps[:, 0:dn], w3[:, 0:2, wsl], h3[:, 0:2, hsl],
                start=True, stop=False,
                perf_mode=mybir.MatmulPerfMode.DoubleRow,
            )
            nc.tensor.matmul(
                ps[:, 0:dn], w3[:, 2:4, wsl], h3[:, 2:4, hsl],
                start=False, stop=False,
                perf_mode=mybir.MatmulPerfMode.DoubleRow,
            )
            nc.tensor.matmul(
                ps[:, 0:dn], w3[:, 4, wsl], h3[:, 4, hsl],
                start=False, stop=True,
            )

        def emit_kproj(dc, jt, pool):
            ps = pool.tile([128, 512], F32, tag="pp", name="pp", bufs=2)
            _proj_mm(ps, k3, ts(dc, 128), ts(jt, 512))
            nc.vector.tensor_copy(kT_sb[dc][:, ts(jt, 512)], ps[:])

        def emit_qproj(dc, it, pool):
            ps = pool.tile([128, 512], F32, tag="pp", name="pp", bufs=2)
            _proj_mm(ps, q3, ts(dc, 128), ts(it, 512))
            nc.vector.tensor_copy(qTz_sb[dc][0][0:DH, ts(it, 512)], ps[0:DH, :])
            nc.vector.tensor_copy(qTz_sb[dc][1][DH:128, ts(it, 512)], ps[DH:128, :])

        def emit_vproj(jc, pool):
            jp, half = divmod(jc, 2)
            vt = v_sb[jp]
            v3 = vt[:, half * VKT : half * VKT + H * VST].rearrange(
                "p (h x) -> p h x", x=VST
            )
            nc.vector.memset(v3[:, :, DH : DH + 1], 1.0)
            for d0, dn in ((0, 512), (512, 128)):
                ps = pool.tile([128, 512], F32, tag="pp", name="pp", bufs=2)
                # stationary = hs chunk pairs (M=128 keys), moving = wv
                nc.tensor.matmul(
                    ps[:, 0:dn], h3[:, 0:2, ts(jc, 128)],
                    v3w[:, 0:2, d0 : d0 + dn],
                    start=True, stop=False,
                    perf_mode=mybir.MatmulPerfMode.DoubleRow,
                )
                nc.tensor.matmul(
                    ps[:, 0:dn], h3[:, 2:4, ts(jc, 128)],
                    v3w[:, 2:4, d0 : d0 + dn],
                    start=False, stop=False,
                    perf_mode=mybir.MatmulPerfMode.DoubleRow,
                )
                nc.tensor.matmul(
                    ps[:, 0:dn], h3[:, 4, ts(jc, 128)],
                    v3w[:, 4, d0 : d0 + dn],
                    start=False, stop=True,
                )
                nc.vector.tensor_copy(
                    v3[:, d0 // DH : (d0 + dn) // DH, 0:DH],
                    ps[:, 0:dn].rearrange("p (h x) -> p h x", x=DH),
                )

        with tc.tile_pool(name="pp0", bufs=2, space="PSUM") as pp0:
            for dc in range(CCH):
                nc.vector.memset(qTz_sb[dc][0][DH:128, :], 0.0)
                nc.vector.memset(qTz_sb[dc][1][0:DH, :], 0.0)
            for jt in range(2):
                emit_kproj(0, jt, pp0)
            for it in range(NIT):
                emit_qproj(0, it, pp0)

        # ---------------- attention phase ----------------
        with tc.tile_pool(name="ap", bufs=1, space="PSUM") as ap, \
             tc.tile_pool(name="pt", bufs=6) as pt_pool, \
             tc.tile_pool(name="ob", bufs=3) as ob, \
             tc.tile_pool(name="scratch", bufs=4) as scratch:

            def norm_dve(hp, par, pv, p_isl):
                # copy numerator + denominator row out of PSUM (releases the
                # pv bank), then approx-reciprocal the denominators on DVE
                # (2.8x faster than InstReciprocal, ~2 ULP).
                raw = scratch.tile([DH, 512], BF16, tag="raw", name="raw")
                nc.vector.tensor_copy(raw[:], pv[0:DH, :])
                den = scratch.tile([DH + 1, 512], F32, tag="den", name="den")
                nc.vector.tensor_copy(den[DH : DH + 1, :], pv[DH : DH + 1, :])
                rec = scratch.tile([DH + 1, 512], BF16, tag="rec", name="rec")
                with nc.allow_low_precision(reason="softmax recip bf16"):
                    nc.vector.reciprocal(rec[DH : DH + 1, :], den[DH : DH + 1, :])
                return (hp, par, p_isl, raw, rec)

            def norm_mul(state, anchor=None):
                # rank-1 PE outer product broadcasts the reciprocal row
                # across partitions into a transient pp bank; DVE multiplies.
                # The tile scheduler orders by data deps, which would place
                # this right after the pair's last PV where it blocks the
                # strict-FIFO PE for the full ~3.4us reciprocal; pin it
                # behind a mid-next-pair QK instead.
                hp, par, p_isl, raw, rec = state
                rb = ap.tile([128, 512], F32, tag="pp", bufs=2, name="pp")
                r_mm = nc.tensor.matmul(
                    rb[0:DH, :],
                    ones_sb[DH : DH + 1, :],
                    rec[DH : DH + 1, :],
                    start=True,
                    stop=True,
                )
                if anchor is not None:
                    tile.add_dep_helper(
                        r_mm.ins, anchor.ins, sync=False,
                        reason="norm rank1 after anchor QK (hide recip latency)",
                    )
                nc.vector.tensor_mul(
                    attn_sb[hp][par * DH : (par + 1) * DH, p_isl],
                    raw[:],
                    rb[0:DH, :],
                )

            def emit_oproj(ec, it):
                wos = []
                for hp in range(CCH):
                    wt = ob.tile([128, 128], BF16, tag="woec", name="woec",
                                 bufs=25)
                    nc.sync.dma_start(wt[:], woT[ts(hp, 128), ts(ec, 128)])
                    wos.append(wt)
                ps = ap.tile([128, 512], F32, tag="pp", name="pp", bufs=2)
                for hp in range(CCH):
                    nc.tensor.matmul(
                        ps[:],
                        wos[hp][:],
                        attn_sb[hp][:, ts(it, 512)],
                        start=(hp == 0),
                        stop=(hp == CCH - 1),
                    )
                rt = ob.tile([128, 512], F32, tag="rt", name="rt", bufs=2)
                nc.sync.dma_start(rt[:], res[ts(ec, 128), ts(it, 512)])
                ot = ob.tile([128, 512], F32, tag="ot", name="ot", bufs=2)
                nc.vector.tensor_add(ot[:], ps[:], rt[:])
                nc.sync.dma_start(out[ts(ec, 128), ts(it, 512)], ot[:])

            pending = []
            bg = [
                (lambda jt=jt: emit_kproj(0, jt, ap)) for jt in range(2, NJT)
            ]
            for it in range(NIT):
                isl = ts(it, 512)
                for hp in range(CCH):
                    if it == 0 and hp + 1 < CCH:
                        # prefetch next pair's K/Q chunks in the background
                        bg.extend(
                            (lambda jt=jt, dc=hp + 1: emit_kproj(dc, jt, ap))
                            for jt in range(NJT)
                        )
                        bg.extend(
                            (lambda q_it=q_it, dc=hp + 1: emit_qproj(dc, q_it, ap))
                            for q_it in range(NIT)
                        )
                    if it == 1 and hp >= 1:
                        # spread the it0 output projections across the it1
                        # pairs: it1 has no projection prefetch, and a starved
                        # PE micro-idles every slot, making HAM oscillate
                        ecs = [hp - 1] if hp < 4 else [3, 4]
                        bg.extend(
                            (lambda ec=ec: emit_oproj(ec, 0)) for ec in ecs
                        )
                    pvA = ap.tile([DH + 1, 512], F32, tag="pv", bufs=2,
                                  name="pv")
                    pvB = ap.tile([DH + 1, 512], F32, tag="pv", bufs=2,
                                  name="pv")
                    for gi in range(NJP):
                        scA = ap.tile([128, 1024], F32, tag="scA", bufs=1,
                                      name="scA")
                        scB = ap.tile([128, 1024], F32, tag="scB", bufs=1,
                                      name="scB")
                        # full-K QK (zero-padded moving q keeps HAM warm)
                        last_qk = None
                        for k, jc in enumerate((2 * gi, 2 * gi + 1)):
                            nc.tensor.matmul(
                                scA[:, ts(k, 512)],
                                kT_sb[hp][:, ts(jc, 128)],
                                qTz_sb[hp][0][:, isl],
                                start=True,
                                stop=True,
                            )
                            last_qk = nc.tensor.matmul(
                                scB[:, ts(k, 512)],
                                kT_sb[hp][:, ts(jc, 128)],
                                qTz_sb[hp][1][:, isl],
                                start=True,
                                stop=True,
                            )
                        ptA = pt_pool.tile([128, 1024], PT8, tag="pt",
                                           name="pt")
                        ptB = pt_pool.tile([128, 1024], PT8, tag="pt",
                                           name="pt")
                        if gi in DVE_EXP_GROUPS:
                            for pt, sc in ((ptA, scA), (ptB, scB)):
                                nc.vector.tensor_scalar(
                                    pt[:].bitcast(mybir.dt.int8),
                                    sc[:],
                                    EXP_A,
                                    EXP_B,
                                    mybir.AluOpType.mult,
                                    mybir.AluOpType.add,
                                )
                        else:
                            for pt, sc in ((ptA, scA), (ptB, scB)):
                                nc.scalar.activation(
                                    pt[:], sc[:],
                                    mybir.ActivationFunctionType.Exp,
                                    bias=0.0, scale=SCALE,
                                )
                        # pop the previous pair's normalizations late enough
                        # that the ~3.4us DVE reciprocal has finished: the
                        # rank-1 broadcast sits in the strict-FIFO PE queue,
                        # and popping it early stalls every QK behind it
                        if pending and gi == 4:
                            norm_mul(pending.pop(0), last_qk)
                        if pending and gi == 6:
                            norm_mul(pending.pop(0), last_qk)
                        if it == 0 and hp == 0:
                            if gi == 0:
                                emit_hsT_tail()
                            # V projection rides inside the first pair's
                            # window, each chunk pair just ahead of its PV
                            if bg:
                                bg.pop(0)()
                            emit_vproj(2 * gi, ap)
                            emit_vproj(2 * gi + 1, ap)
                        elif bg:
                            bg.pop(0)()
                        # PV: fp8 DoubleRow over the key-chunk pair
                        vv = v_sb[gi][:].rearrange("p (t z) -> p t z", t=2)
                        for par, pv, pt in ((0, pvA, ptA), (1, pvB, ptB)):
                            h = 2 * hp + par
                            nc.tensor.matmul(
                                pv[:],
                                vv[:, :, h * VST : h * VST + VST],
                                pt[:].rearrange("p (t n) -> p t n", t=2),
                                start=(gi == 0),
                                stop=(gi == NJP - 1),
                                perf_mode=mybir.MatmulPerfMode.DoubleRow,
                            )
                    pending.append(norm_dve(hp, 0, pvA, isl))
                    pending.append(norm_dve(hp, 1, pvB, isl))
                    # barrier: next pair's kT/qT must be fully emitted
                    # before its first QK reads them
                    while bg:
                        bg.pop(0)()
            for st in pending:
                norm_mul(st)
            for ec in range(CCH):
                emit_oproj(ec, 1)

    _spill_matmul_waits(nc)
    return nc


# walrus embedded-sync-wait capacity per BIR opcode.  Matmult holds a
# single wait; excess waits hoist onto the paired Ldweights (in-order
# issue on PE makes that equivalent).  Other compute ops spill onto
# EventSemaphore carrier instructions inserted just before them on the
# same engine.  DMACopy / Drain / EventSemaphore handle many waits
# natively (bacc emits such itself) and are left alone.
_WAIT_CAPS = {
    "InstMatmult": 1,
    "InstLdweights": 1,
    "InstActivation": 1,
    "InstReciprocal": 1,
    "InstTensorTensor": 1,
    "InstTensorCopy": 1,
    "InstTensorScalarPtr": 1,
    "InstTensorReduce": 1,
    "InstMemset": 1,
    "InstDMACopy": 1,
    "InstDrain": 1,
    "InstCustomDveAnt": 1,
}
_ES_CAP = 2  # waits per EventSemaphore carrier (walrus: <=2 waits, <=1 update)


def _spill_matmul_waits(nc: bass.Bass) -> None:
    spill_id = [0]

    def carriers(excess, engine):
        out = []
        for i in range(0, len(excess), _ES_CAP):
            es = mybir.InstEventSemaphore(
                name=f"wait-spill-{spill_id[0]}", ins=[], outs=[]
            )
            spill_id[0] += 1
            es.engine = engine
            es.sync_info = mybir.SyncInfo(
                on_wait=excess[i : i + _ES_CAP], on_update=[]
            )
            out.append(es)
        return out

    for f in nc.m.functions:
        for blk in f.blocks:
            insts = blk.instructions
            i = 0
            while i < len(insts):
                inst = insts[i]
                tn = type(inst).__name__
                cap = _WAIT_CAPS.get(tn)
                si = inst.sync_info
                if cap is None or si is None or len(si.on_wait) <= cap:
                    i += 1
                    continue
                w = list(si.on_wait)
                if tn == "InstMatmult" and cap == 1:
                    # Keep the latest-satisfied dependency (the ACT-produced
                    # operand, e.g. probs from exp) embedded on the matmul and
                    # hoist early ones onto the Ldweights: a wait on the LDW
                    # blocks its background prefetch and serializes ~50ns of
                    # weight-load into every PV matmul.
                    acts = [x for x in w if "Activation" in (x.ant_name or "")]
                    if acts:
                        keep = [acts[-1]]
                        excess = [x for x in w if x is not acts[-1]]
                    else:
                        keep, excess = w[-cap:], w[:-cap]
                else:
                    keep, excess = w[-cap:], w[:-cap]
                prev = insts[i - 1] if i > 0 else None
                if (
                    tn == "InstMatmult"
                    and prev is not None
                    and type(prev).__name__ == "InstLdweights"
                    and len(((prev.sync_info and prev.sync_info.on_wait) or []))
                    + len(excess) <= 1
                ):
                    psi = prev.sync_info
                    pw = list(psi.on_wait) if psi is not None else []
                    pu = list(psi.on_update) if psi is not None else []
                    prev.sync_info = mybir.SyncInfo(on_wait=pw + excess, on_update=pu)
                else:
                    new = carriers(excess, inst.engine)
                    insts[i:i] = new
                    i += len(new)
                inst.sync_info = mybir.SyncInfo(
                    on_wait=keep, on_update=list(si.on_update)
                )
                i += 1


_CACHED_NC = None


def get_nc() -> bass.Bass:
    global _CACHED_NC
    if _CACHED_NC is None:
        _CACHED_NC = build_nc()
    return _CACHED_NC


def _chunk_major(mT, n, f8):
    # [C, n] -> [128, CCH*n] fp8: row p, col cc*n+j = mT[128*cc + p, j]
    return np.ascontiguousarray(
        mT.reshape(CCH, 128, n).transpose(1, 0, 2).reshape(128, CCH * n)
    ).astype(f8)


def make_in_maps(hidden_states, Wq, Wk, Wv, Wo, b_out):
    hs = np.asarray(hidden_states, dtype=np.float32)
    bf = ml_dtypes.bfloat16
    f8 = mybir.dt.np(FP8)
    wq2 = _chunk_major(np.asarray(Wq, np.float32).T, C, f8)
    wk2 = _chunk_major(np.asarray(Wk, np.float32).T, C, f8)
    wv2 = _chunk_major(np.asarray(Wv, np.float32).T, C, f8)
    woT = np.ascontiguousarray(np.asarray(Wo, np.float32).T).astype(bf)
    bias = np.asarray(b_out, np.float32).reshape(C, 1)
    in_maps = []
    for c in range(NCORES):
        b, g = divmod(c, GROUP)
        i0 = g * SQ
        hsTb = hs[b].T  # [C, S]
        in_maps.append(
            {
                "hs2": _chunk_major(np.roll(hsTb, -i0, axis=1), S, f8),
                "res": np.ascontiguousarray(hsTb[:, i0 : i0 + SQ]) + bias,
                "wq2": wq2,
                "wk2": wk2,
                "wv2": wv2,
                "woT": woT,
            }
        )
    return in_maps


def assemble(results) -> np.ndarray:
    y = np.empty((B, S, C), np.float32)
    for c in range(NCORES):
        b, g = divmod(c, GROUP)
        i0 = g * SQ
        y[b, i0 : i0 + SQ, :] = np.asarray(results[c]["out"], np.float32).T
    return y


def kernel(**inputs) -> np.ndarray:
    from concourse.bass_utils import run_bass_kernel_spmd

    nc = get_nc()
    in_maps = make_in_maps(**inputs)
    res = run_bass_kernel_spmd(nc, in_maps, list(range(NCORES)))
    return assemble(res.results)


if __name__ == "__main__":
    import reference

    inputs = {k: np.asarray(v) for k, v in reference.setup_inputs().items()}
    got = kernel(**inputs)
    want = np.asarray(reference.reference(**inputs))
    err = np.linalg.norm(got - want) / np.linalg.norm(want)
    print("Relative error:", err)


# revision 37
# speedup vs baseline: 1.0735x; 1.0693x over previous
"""Multi-head attention (AttnProcessor2_0) on 8 TRN2 NeuronCores.

Problem: B=2, S=4096, C=640, H=10, Dh=64.
  q/k/v = hs @ W{q,k,v}.T ; per-head scores = q k^T / 8 ; softmax ;
  out = probs v ; y = out @ Wo.T + b_out + hs

Sharding (no collectives): core c -> batch b=c//4, query block g=c%4
(1024 queries).  Each core recomputes full K/V for its batch (head-dim
on partitions), computes its own S/4 x S attention block, output
projection, bias+residual.  Host passes hidden states TRANSPOSED and
ROLLED by the query offset so the same SPMD program works on every
core (softmax+PV are permutation-invariant along the key axis).

v3 layout (vs the 514us baseline):
  * PV fp8 DoubleRow: probs written by ScalarE exp directly as fp8
    (e5m2 -- e4m3 stores cost ScalarE ~20% extra, e5m2 runs at bf16
    rate), v projected to fp8e4 with key-chunk PAIRS packed at stride
    656; each DR matmul contracts 256 keys (moving free dim 1024 >=
    256 so the ~1.5x DR win applies).  Denominators still fall out as
    PSUM row 64 via the ones column.  PV: 327,680 -> 163,840 cycles.
  * Q/K/V projections in fp8 DoubleRow: hs and Wq/Wk/Wv uploaded as
    fp8e4 in chunk-major [128, chunk, n] layout so DR pairs feature
    chunks along the free axis; 640-contraction = 2 DR + 1 plain
    matmul (5 -> 3 instructions per tile).
  * O proj de-padded: attention outputs are packed per head-PAIR
    [128, SQ], so the 640-contraction runs dense (25,600 cycles).
  * Normalization off the pv critical path: numerator+denominator rows
    are copied out of PSUM immediately (releasing the bank), the bf16
    rank-1 reciprocal broadcast goes to a transient pp bank, and the
    pending broadcasts pop at gi 5/7 of the NEXT pair so the ~3.4us
    DVE reciprocal never stalls the strict-FIFO PE queue.
  * QK stays zero-padded full-K (K=64 matmuls or 64-row tiles drop
    HAM's utilization accounting below its limit and the PE is
    clock-gated to 1.2 GHz).  PSUM: 4 banks scores (A/B ping-pong) +
    2 pv + 2 background-projection banks.
"""

import sys

if "/opt/trn_rl_repo" not in sys.path:
    sys.path.insert(0, "/opt/trn_rl_repo")

from contextlib import ExitStack

import ml_dtypes
import numpy as np

import concourse.bass as bass
import concourse.tile as tile
from concourse import mybir
from concourse.bass import ts

BF16 = mybir.dt.bfloat16
F32 = mybir.dt.float32
FP8 = mybir.dt.float8e4  # v tiles (e4m3: 3-bit mantissa)
PT8 = mybir.dt.float8e5  # probs from exp (e5m2)

B, S, C = 2, 4096, 640
H, DH = 10, 64
NCORES = 8
GROUP = 4  # cores per batch element
SQ = S // GROUP  # 1024 queries per core
SCALE = 0.125  # 1/sqrt(64)
CCH = C // 128  # 5 feature chunks (2 heads each)
NJT = S // 512  # 8 key tiles for K proj
NJC = S // 128  # 32 key chunks for attention
NJP = NJC // 2  # 16 key chunk PAIRS (DoubleRow k-tiles)
NIT = SQ // 512  # 2 query tiles
VST = DH + 1  # 65: per-head stride in v tiles (ones col at f=64)
VKT = 656  # per-ktile stride in v pair tiles (10*65=650 padded to %16==0)

# Schraudolph exp-as-int8 on DVE: writing round(A*score + B) as int8 and
# bitcasting the bytes as e5m2 IS exp(score*SCALE) to within ~3% (the
# mantissa-linear 2^f approximation) -- same order as the e5m2 grid the
# ScalarE path rounds onto anyway.  Offloading a fixed subset of exp
# groups to DVE relieves the ScalarE bottleneck (the only engine with a
# real exp) and keeps the PE the busiest engine, which also keeps HAM's
# activity monitor from clock-gating it.  B folds the e5m2 bias (15) and
# the half-max 2^f correction (-0.0436).
EXP_A = 4.0 * 1.4426950408889634 * SCALE
EXP_B = 4.0 * (15.0 - 0.0436)
# measured: DVE int8 stores run at HALF rate (2.1us per 1024-col call vs
# ScalarE's 1.11us) -- the offload only pays if DVE is otherwise idle,
# which it is not.  Disabled; ScalarE keeps all exp groups.
DVE_EXP_GROUPS = ()


def build_nc() -> bass.Bass:
    nc = bass.Bass()
    # hs and the q/k/v weights come in as fp8e4 in chunk-major layout
    # [128, chunk, n] so DoubleRow can pair feature chunks along the free
    # axis (the two k-tiles of a DR matmul must share partitions).
    hs2 = nc.declare_dram_parameter("hs2", [128, CCH * S], FP8, isOutput=False)
    res = nc.declare_dram_parameter("res", [C, SQ], F32, isOutput=False)
    wq2 = nc.declare_dram_parameter("wq2", [128, CCH * C], FP8, isOutput=False)
    wk2 = nc.declare_dram_parameter("wk2", [128, CCH * C], FP8, isOutput=False)
    wv2 = nc.declare_dram_parameter("wv2", [128, CCH * C], FP8, isOutput=False)
    woT = nc.declare_dram_parameter("woT", [C, C], BF16, isOutput=False)
    out = nc.declare_dram_parameter("out", [C, SQ], F32, isOutput=True)

    with ExitStack() as ctx:
        tc = ctx.enter_context(tile.TileContext(nc))
        # outer pool: tensors whose lifetime spans projections AND attention
        sb = ctx.enter_context(tc.tile_pool(name="sb", bufs=1))

        kT_sb = [sb.tile([128, S], BF16, tag=f"kT{i}", name=f"kT{i}") for i in range(CCH)]
        # per-head q, zero-padded to full 128-row contraction: partial-K
        # (K=64) matmuls drop HAM's utilization accounting below its 0.5
        # limit and clock the PE down to 1.2 GHz -- padding the contraction
        # with zero rows is exact and keeps the array at the warm rate.
        qTz_sb = [
            [sb.tile([128, SQ], BF16, tag=f"qz{i}_{p}", name=f"qz{i}_{p}")
             for p in range(2)]
            for i in range(CCH)
        ]
        # v: fp8, key-chunk pairs side by side (ktile stride 656 for DR)
        v_sb = [sb.tile([128, 2 * VKT], FP8, tag=f"v{j}", name=f"v{j}") for j in range(NJP)]
        # attention outputs packed per head pair (no zero padding)
        attn_sb = [sb.tile([128, SQ], BF16, tag=f"attn{h}", name=f"attn{h}")
                   for h in range(CCH)]
        ones_sb = sb.tile([128, DH], BF16, tag="ones", name="ones")
        nc.vector.memset(ones_sb[:], 1.0)

        # ---------------- load + first projections ----------------
        load = ctx.enter_context(tc.tile_pool(name="load", bufs=1))
        hs_sb = load.tile([128, CCH * S], FP8, tag="hs2", name="hs2")
        wk_sb = load.tile([128, CCH * C], FP8, tag="wk2", name="wk2")
        wq_sb = load.tile([128, CCH * C], FP8, tag="wq2", name="wq2")
        wv_sb = load.tile([128, CCH * C], FP8, tag="wv2", name="wv2")
        nc.sync.dma_start(wk_sb[:], wk2[:, :])
        for cc in range(CCH):
            nc.sync.dma_start(
                hs_sb[:, cc * S : cc * S + SQ], hs2[:, cc * S : cc * S + SQ]
            )
        nc.sync.dma_start(wq_sb[:], wq2[:, :])
        nc.sync.dma_start(wv_sb[:], wv2[:, :])
        h3 = hs_sb[:].rearrange("p (c s) -> p c s", c=CCH)
        k3 = wk_sb[:].rearrange("p (c f) -> p c f", c=CCH)
        q3 = wq_sb[:].rearrange("p (c f) -> p c f", c=CCH)
        v3w = wv_sb[:].rearrange("p (c f) -> p c f", c=CCH)

        def emit_hsT_tail():
            # deferred until after the first exp so ScalarE's conservative
            # vector-clock waits don't cover this 2MB of DMA
            for blk in range(SQ, S, SQ):
                for cc in range(CCH):
                    nc.sync.dma_start(
                        hs_sb[:, cc * S + blk : cc * S + blk + SQ],
                        hs2[:, cc * S + blk : cc * S + blk + SQ],
                    )

        def _proj_mm(ps, w3, wsl, hsl, dn=512):
            # contraction over 640 = 2 fp8 DoubleRow k-tile pairs + 1 plain
            nc.tensor.matmul(
                ps[:, 0:dn], w3[:, 0:2, wsl], h3[:, 0:2, hsl],
                start=True, stop=False,
                perf_mode=mybir.MatmulPerfMode.DoubleRow,
            )
            nc.tensor.matmul(
                ps[:, 0:dn], w3[:, 2:4, wsl], h3[:, 2:4, hsl],
                start=False, stop=False,
                perf_mode=mybir.MatmulPerfMode.DoubleRow,
            )
            nc.tensor.matmul(
                ps[:, 0:dn], w3[:, 4, wsl], h3[:, 4, hsl],
                start=False, stop=True,
            )

        def emit_kproj(dc, jt, pool):
            ps = pool.tile([128, 512], F32, tag="pp", name="pp", bufs=2)
            _proj_mm(ps, k3, ts(dc, 128), ts(jt, 512))
            nc.vector.tensor_copy(kT_sb[dc][:, ts(jt, 512)], ps[:])

        def emit_qproj(dc, it, pool):
            ps = pool.tile([128, 512], F32, tag="pp", name="pp", bufs=2)
            _proj_mm(ps, q3, ts(dc, 128), ts(it, 512))
            nc.vector.tensor_copy(qTz_sb[dc][0][0:DH, ts(it, 512)], ps[0:DH, :])
            nc.vector.tensor_copy(qTz_sb[dc][1][DH:128, ts(it, 512)], ps[DH:128, :])

        def emit_vproj(jc, pool):
            jp, half = divmod(jc, 2)
            vt = v_sb[jp]
            v3 = vt[:, half * VKT : half * VKT + H * VST].rearrange(
                "p (h x) -> p h x", x=VST
            )
            nc.vector.memset(v3[:, :, DH : DH + 1], 1.0)
            for d0, dn in ((0, 512), (512, 128)):
                ps = pool.tile([128, 512], F32, tag="pp", name="pp", bufs=2)
                # stationary = hs chunk pairs (M=128 keys), moving = wv
                nc.tensor.matmul(
                    ps[:, 0:dn], h3[:, 0:2, ts(jc, 128)],
                    v3w[:, 0:2, d0 : d0 + dn],
                    start=True, stop=False,
                    perf_mode=mybir.MatmulPerfMode.DoubleRow,
                )
                nc.tensor.matmul(
                    ps[:, 0:dn], h3[:, 2:4, ts(jc, 128)],
                    v3w[:, 2:4, d0 : d0 + dn],
                    start=False, stop=False,
                    perf_mode=mybir.MatmulPerfMode.DoubleRow,
                )
                nc.tensor.matmul(
                    ps[:, 0:dn], h3[:, 4, ts(jc, 128)],
                    v3w[:, 4, d0 : d0 + dn],
                    start=False, stop=True,
                )
                nc.vector.tensor_copy(
                    v3[:, d0 // DH : (d0 + dn) // DH, 0:DH],
                    ps[:, 0:dn].rearrange("p (h x) -> p h x", x=DH),
                )

        with tc.tile_pool(name="pp0", bufs=2, space="PSUM") as pp0:
            for dc in range(CCH):
                nc.vector.memset(qTz_sb[dc][0][DH:128, :], 0.0)
                nc.vector.memset(qTz_sb[dc][1][0:DH, :], 0.0)
            for jt in range(2):
                emit_kproj(0, jt, pp0)
            for it in range(NIT):
                emit_qproj(0, it, pp0)

        # ---------------- attention phase ----------------
        with tc.tile_pool(name="ap", bufs=1, space="PSUM") as ap, \
             tc.tile_pool(name="pt", bufs=6) as pt_pool, \
             tc.tile_pool(name="ob", bufs=3) as ob, \
             tc.tile_pool(name="scratch", bufs=4) as scratch:

            def norm_dve(hp, par, pv, p_isl):
                # copy numerator + denominator row out of PSUM (releases the
                # pv bank), then approx-reciprocal the denominators on DVE
                # (2.8x faster than InstReciprocal, ~2 ULP).
                raw = scratch.tile([DH, 512], BF16, tag="raw", name="raw")
                nc.vector.tensor_copy(raw[:], pv[0:DH, :])
                den = scratch.tile([DH + 1, 512], F32, tag="den", name="den")
                nc.vector.tensor_copy(den[DH : DH + 1, :], pv[DH : DH + 1, :])
                rec = scratch.tile([DH + 1, 512], BF16, tag="rec", name="rec")
                with nc.allow_low_precision(reason="softmax recip bf16"):
                    nc.vector.reciprocal(rec[DH : DH + 1, :], den[DH : DH + 1, :])
                return (hp, par, p_isl, raw, rec)

            def norm_mul(state, anchor=None):
                # rank-1 PE outer product broadcasts the reciprocal row
                # across partitions into a transient pp bank; DVE multiplies.
                # The tile scheduler orders by data deps, which would place
                # this right after the pair's last PV where it blocks the
                # strict-FIFO PE for the full ~3.4us reciprocal; pin it
                # behind a mid-next-pair QK instead.
                hp, par, p_isl, raw, rec = state
                rb = ap.tile([128, 512], F32, tag="pp", bufs=2, name="pp")
                r_mm = nc.tensor.matmul(
                    rb[0:DH, :],
                    ones_sb[DH : DH + 1, :],
                    rec[DH : DH + 1, :],
                    start=True,
                    stop=True,
                )
                if anchor is not None:
                    tile.add_dep_helper(
                        r_mm.ins, anchor.ins, sync=False,
                        reason="norm rank1 after anchor QK (hide recip latency)",
                    )
                nc.vector.tensor_mul(
                    attn_sb[hp][par * DH : (par + 1) * DH, p_isl],
                    raw[:],
                    rb[0:DH, :],
                )

            def emit_oproj(ec, it):
                wos = []
                for hp in range(CCH):
                    wt = ob.tile([128, 128], BF16, tag="woec", name="woec",
                                 bufs=25)
                    nc.sync.dma_start(wt[:], woT[ts(hp, 128), ts(ec, 128)])
                    wos.append(wt)
                ps = ap.tile([128, 512], F32, tag="pp", name="pp", bufs=2)
                for hp in range(CCH):
                    nc.tensor.matmul(
                        ps[:],
                        wos[hp][:],
                        attn_sb[hp][:, ts(it, 512)],
                        start=(hp == 0),
                        stop=(hp == CCH - 1),
                    )
                rt = ob.tile([128, 512], F32, tag="rt", name="rt", bufs=2)
                nc.sync.dma_start(rt[:], res[ts(ec, 128), ts(it, 512)])
                ot = ob.tile([128, 512], F32, tag="ot", name="ot", bufs=2)
                nc.vector.tensor_add(ot[:], ps[:], rt[:])
                nc.sync.dma_start(out[ts(ec, 128), ts(it, 512)], ot[:])

            pending = []
            # the last group's PVs of each pair are deferred into the next
            # pair, pinned behind its first QK: otherwise the chain
            # last-exp -> last-PV -> first-QK -> first-exp leaves ScalarE
            # idle ~3us at every pair boundary
            defer = [None]

            def flush_defer(anchor):
                d_hp, d_pvA, d_pvB, d_ptA, d_ptB, d_isl = defer[0]
                vv = v_sb[NJP - 1][:].rearrange("p (t z) -> p t z", t=2)
                for par, pv, pt in ((0, d_pvA, d_ptA), (1, d_pvB, d_ptB)):
                    h = 2 * d_hp + par
                    mm = nc.tensor.matmul(
                        pv[:],
                        vv[:, :, h * VST : h * VST + VST],
                        pt[:].rearrange("p (t n) -> p t n", t=2),
                        start=False,
                        stop=True,
                        perf_mode=mybir.MatmulPerfMode.DoubleRow,
                    )
                    if anchor is not None:
                        tile.add_dep_helper(
                            mm.ins, anchor.ins, sync=False,
                            reason="deferred last PV behind next pair's QK",
                        )
                pending.append(norm_dve(d_hp, 0, d_pvA, d_isl))
                pending.append(norm_dve(d_hp, 1, d_pvB, d_isl))
                defer[0] = None

            bg = [
                (lambda jt=jt: emit_kproj(0, jt, ap)) for jt in range(2, NJT)
            ]
            for it in range(NIT):
                isl = ts(it, 512)
                for hp in range(CCH):
                    if it == 0 and hp + 1 < CCH:
                        # prefetch next pair's K/Q chunks in the background
                        bg.extend(
                            (lambda jt=jt, dc=hp + 1: emit_kproj(dc, jt, ap))
                            for jt in range(NJT)
                        )
                        bg.extend(
                            (lambda q_it=q_it, dc=hp + 1: emit_qproj(dc, q_it, ap))
                            for q_it in range(NIT)
                        )
                    if it == 1 and hp >= 1:
                        # spread the it0 output projections across the it1
                        # pairs: it1 has no projection prefetch, and a starved
                        # PE micro-idles every slot, making HAM oscillate
                        ecs = [hp - 1] if hp < 4 else [3, 4]
                        bg.extend(
                            (lambda ec=ec: emit_oproj(ec, 0)) for ec in ecs
                        )
                    pvA = ap.tile([DH + 1, 512], F32, tag="pv", bufs=2,
                                  name="pv")
                    pvB = ap.tile([DH + 1, 512], F32, tag="pv", bufs=2,
                                  name="pv")
                    for gi in range(NJP):
                        scA = ap.tile([128, 1024], F32, tag="scA", bufs=1,
                                      name="scA")
                        scB = ap.tile([128, 1024], F32, tag="scB", bufs=1,
                                      name="scB")
                        # full-K QK (zero-padded moving q keeps HAM warm)
                        last_qk = None
                        for k, jc in enumerate((2 * gi, 2 * gi + 1)):
                            nc.tensor.matmul(
                                scA[:, ts(k, 512)],
                                kT_sb[hp][:, ts(jc, 128)],
                                qTz_sb[hp][0][:, isl],
                                start=True,
                                stop=True,
                            )
                            last_qk = nc.tensor.matmul(
                                scB[:, ts(k, 512)],
                                kT_sb[hp][:, ts(jc, 128)],
                                qTz_sb[hp][1][:, isl],
                                start=True,
                                stop=True,
                            )
                        if gi == 0 and defer[0] is not None:
                            flush_defer(last_qk)
                        ptA = pt_pool.tile([128, 1024], PT8, tag="pt",
                                           name="pt")
                        ptB = pt_pool.tile([128, 1024], PT8, tag="pt",
                                           name="pt")
                        if gi in DVE_EXP_GROUPS:
                            for pt, sc in ((ptA, scA), (ptB, scB)):
                                nc.vector.tensor_scalar(
                                    pt[:].bitcast(mybir.dt.int8),
                                    sc[:],
                                    EXP_A,
                                    EXP_B,
                                    mybir.AluOpType.mult,
                                    mybir.AluOpType.add,
                                )
                        else:
                            for pt, sc in ((ptA, scA), (ptB, scB)):
                                nc.scalar.activation(
                                    pt[:], sc[:],
                                    mybir.ActivationFunctionType.Exp,
                                    bias=0.0, scale=SCALE,
                                )
                        # pop the previous pair's normalizations late enough
                        # that the ~3.4us DVE reciprocal has finished: the
                        # rank-1 broadcast sits in the strict-FIFO PE queue,
                        # and popping it early stalls every QK behind it
                        if pending and gi == 4:
                            norm_mul(pending.pop(0), last_qk)
                        if pending and gi == 6:
                            norm_mul(pending.pop(0), last_qk)
                        if it == 0 and hp == 0:
                            if gi == 0:
                                emit_hsT_tail()
                            # V projection rides inside the first pair's
                            # window, each chunk pair just ahead of its PV
                            if bg:
                                bg.pop(0)()
                            emit_vproj(2 * gi, ap)
                            emit_vproj(2 * gi + 1, ap)
                        elif bg:
                            bg.pop(0)()
                        # PV: fp8 DoubleRow over the key-chunk pair (the
                        # final group is deferred into the next pair)
                        if gi == NJP - 1:
                            defer[0] = (hp, pvA, pvB, ptA, ptB, isl)
                        else:
                            vv = v_sb[gi][:].rearrange("p (t z) -> p t z", t=2)
                            for par, pv, pt in ((0, pvA, ptA), (1, pvB, ptB)):
                                h = 2 * hp + par
                                nc.tensor.matmul(
                                    pv[:],
                                    vv[:, :, h * VST : h * VST + VST],
                                    pt[:].rearrange("p (t n) -> p t n", t=2),
                                    start=(gi == 0),
                                    stop=False,
                                    perf_mode=mybir.MatmulPerfMode.DoubleRow,
                                )
                    # barrier: next pair's kT/qT must be fully emitted
                    # before its first QK reads them
                    while bg:
                        bg.pop(0)()
            if defer[0] is not None:
                flush_defer(None)
            for st in pending:
                norm_mul(st)
            for ec in range(CCH):
                emit_oproj(ec, 1)

    _spill_matmul_waits(nc)
    return nc


# walrus embedded-sync-wait capacity per BIR opcode.  Matmult holds a
# single wait; excess waits hoist onto the paired Ldweights (in-order
# issue on PE makes that equivalent).  Other compute ops spill onto
# EventSemaphore carrier instructions inserted just before them on the
# same engine.  DMACopy / Drain / EventSemaphore handle many waits
# natively (bacc emits such itself) and are left alone.
_WAIT_CAPS = {
    "InstMatmult": 1,
    "InstLdweights": 1,
    "InstActivation": 1,
    "InstReciprocal": 1,
    "InstTensorTensor": 1,
    "InstTensorCopy": 1,
    "InstTensorScalarPtr": 1,
    "InstTensorReduce": 1,
    "InstMemset": 1,
    "InstDMACopy": 1,
    "InstDrain": 1,
    "InstCustomDveAnt": 1,
}
_ES_CAP = 2  # waits per EventSemaphore carrier (walrus: <=2 waits, <=1 update)


def _spill_matmul_waits(nc: bass.Bass) -> None:
    spill_id = [0]

    def carriers(excess, engine):
        out = []
        for i in range(0, len(excess), _ES_CAP):
            es = mybir.InstEventSemaphore(
                name=f"wait-spill-{spill_id[0]}", ins=[], outs=[]
            )
            spill_id[0] += 1
            es.engine = engine
            es.sync_info = mybir.SyncInfo(
                on_wait=excess[i : i + _ES_CAP], on_update=[]
            )
            out.append(es)
        return out

    for f in nc.m.functions:
        for blk in f.blocks:
            insts = blk.instructions
            i = 0
            while i < len(insts):
                inst = insts[i]
                tn = type(inst).__name__
                cap = _WAIT_CAPS.get(tn)
                si = inst.sync_info
                if cap is None or si is None or len(si.on_wait) <= cap:
                    i += 1
                    continue
                w = list(si.on_wait)
                if tn == "InstMatmult" and cap == 1:
                    # Keep the latest-satisfied dependency (the ACT-produced
                    # operand, e.g. probs from exp) embedded on the matmul and
                    # hoist early ones onto the Ldweights: a wait on the LDW
                    # blocks its background prefetch and serializes ~50ns of
                    # weight-load into every PV matmul.
                    acts = [x for x in w if "Activation" in (x.ant_name or "")]
                    if acts:
                        keep = [acts[-1]]
                        excess = [x for x in w if x is not acts[-1]]
                    else:
                        keep, excess = w[-cap:], w[:-cap]
                else:
                    keep, excess = w[-cap:], w[:-cap]
                prev = insts[i - 1] if i > 0 else None
                if (
                    tn == "InstMatmult"
                    and prev is not None
                    and type(prev).__name__ == "InstLdweights"
                    and len(((prev.sync_info and prev.sync_info.on_wait) or []))
                    + len(excess) <= 1
                ):
                    psi = prev.sync_info
                    pw = list(psi.on_wait) if psi is not None else []
                    pu = list(psi.on_update) if psi is not None else []
                    prev.sync_info = mybir.SyncInfo(on_wait=pw + excess, on_update=pu)
                else:
                    new = carriers(excess, inst.engine)
                    insts[i:i] = new
                    i += len(new)
                inst.sync_info = mybir.SyncInfo(
                    on_wait=keep, on_update=list(si.on_update)
                )
                i += 1


_CACHED_NC = None


def get_nc() -> bass.Bass:
    global _CACHED_NC
    if _CACHED_NC is None:
        _CACHED_NC = build_nc()
    return _CACHED_NC


def _chunk_major(mT, n, f8):
    # [C, n] -> [128, CCH*n] fp8: row p, col cc*n+j = mT[128*cc + p, j]
    return np.ascontiguousarray(
        mT.reshape(CCH, 128, n).transpose(1, 0, 2).reshape(128, CCH * n)
    ).astype(f8)


def make_in_maps(hidden_states, Wq, Wk, Wv, Wo, b_out):
    hs = np.asarray(hidden_states, dtype=np.float32)
    bf = ml_dtypes.bfloat16
    f8 = mybir.dt.np(FP8)
    wq2 = _chunk_major(np.asarray(Wq, np.float32).T, C, f8)
    wk2 = _chunk_major(np.asarray(Wk, np.float32).T, C, f8)
    wv2 = _chunk_major(np.asarray(Wv, np.float32).T, C, f8)
    woT = np.ascontiguousarray(np.asarray(Wo, np.float32).T).astype(bf)
    bias = np.asarray(b_out, np.float32).reshape(C, 1)
    in_maps = []
    for c in range(NCORES):
        b, g = divmod(c, GROUP)
        i0 = g * SQ
        hsTb = hs[b].T  # [C, S]
        in_maps.append(
            {
                "hs2": _chunk_major(np.roll(hsTb, -i0, axis=1), S, f8),
                "res": np.ascontiguousarray(hsTb[:, i0 : i0 + SQ]) + bias,
                "wq2": wq2,
                "wk2": wk2,
                "wv2": wv2,
                "woT": woT,
            }
        )
    return in_maps


def assemble(results) -> np.ndarray:
    y = np.empty((B, S, C), np.float32)
    for c in range(NCORES):
        b, g = divmod(c, GROUP)
        i0 = g * SQ
        y[b, i0 : i0 + SQ, :] = np.asarray(results[c]["out"], np.float32).T
    return y


def kernel(**inputs) -> np.ndarray:
    from concourse.bass_utils import run_bass_kernel_spmd

    nc = get_nc()
    in_maps = make_in_maps(**inputs)
    res = run_bass_kernel_spmd(nc, in_maps, list(range(NCORES)))
    return assemble(res.results)


if __name__ == "__main__":
    import reference

    inputs = {k: np.asarray(v) for k, v in reference.setup_inputs().items()}
    got = kernel(**inputs)
    want = np.asarray(reference.reference(**inputs))
    err = np.linalg.norm(got - want) / np.linalg.norm(want)
    print("Relative error:", err)


# revision 38
# speedup vs baseline: 1.0793x; 1.0054x over previous
"""Multi-head attention (AttnProcessor2_0) on 8 TRN2 NeuronCores.

Problem: B=2, S=4096, C=640, H=10, Dh=64.
  q/k/v = hs @ W{q,k,v}.T ; per-head scores = q k^T / 8 ; softmax ;
  out = probs v ; y = out @ Wo.T + b_out + hs

Sharding (no collectives): core c -> batch b=c//4, query block g=c%4
(1024 queries).  Each core recomputes full K/V for its batch (head-dim
on partitions), computes its own S/4 x S attention block, output
projection, bias+residual.  Host passes hidden states TRANSPOSED and
ROLLED by the query offset so the same SPMD program works on every
core (softmax+PV are permutation-invariant along the key axis).

v3 layout (vs the 514us baseline):
  * PV fp8 DoubleRow: probs written by ScalarE exp directly as fp8
    (e5m2 -- e4m3 stores cost ScalarE ~20% extra, e5m2 runs at bf16
    rate), v projected to fp8e4 with key-chunk PAIRS packed at stride
    656; each DR matmul contracts 256 keys (moving free dim 1024 >=
    256 so the ~1.5x DR win applies).  Denominators still fall out as
    PSUM row 64 via the ones column.  PV: 327,680 -> 163,840 cycles.
  * Q/K/V projections in fp8 DoubleRow: hs and Wq/Wk/Wv uploaded as
    fp8e4 in chunk-major [128, chunk, n] layout so DR pairs feature
    chunks along the free axis; 640-contraction = 2 DR + 1 plain
    matmul (5 -> 3 instructions per tile).
  * O proj de-padded: attention outputs are packed per head-PAIR
    [128, SQ], so the 640-contraction runs dense (25,600 cycles).
  * Normalization off the pv critical path: numerator+denominator rows
    are copied out of PSUM immediately (releasing the bank), the bf16
    rank-1 reciprocal broadcast goes to a transient pp bank, and the
    pending broadcasts pop at gi 5/7 of the NEXT pair so the ~3.4us
    DVE reciprocal never stalls the strict-FIFO PE queue.
  * QK stays zero-padded full-K (K=64 matmuls or 64-row tiles drop
    HAM's utilization accounting below its limit and the PE is
    clock-gated to 1.2 GHz).  PSUM: 4 banks scores (A/B ping-pong) +
    2 pv + 2 background-projection banks.
"""

import sys

if "/opt/trn_rl_repo" not in sys.path:
    sys.path.insert(0, "/opt/trn_rl_repo")

from contextlib import ExitStack

import ml_dtypes
import numpy as np

import concourse.bass as bass
import concourse.tile as tile
from concourse import mybir
from concourse.bass import ts

BF16 = mybir.dt.bfloat16
F32 = mybir.dt.float32
FP8 = mybir.dt.float8e4  # v tiles (e4m3: 3-bit mantissa)
PT8 = mybir.dt.float8e5  # probs from exp (e5m2)

B, S, C = 2, 4096, 640
H, DH = 10, 64
NCORES = 8
GROUP = 4  # cores per batch element
SQ = S // GROUP  # 1024 queries per core
SCALE = 0.125  # 1/sqrt(64)
CCH = C // 128  # 5 feature chunks (2 heads each)
NJT = S // 512  # 8 key tiles for K proj
NJC = S // 128  # 32 key chunks for attention
NJP = NJC // 2  # 16 key chunk PAIRS (DoubleRow k-tiles)
NIT = SQ // 512  # 2 query tiles
VST = DH + 1  # 65: per-head stride in v tiles (ones col at f=64)
VKT = 656  # per-ktile stride in v pair tiles (10*65=650 padded to %16==0)

# Schraudolph exp-as-int8 on DVE: writing round(A*score + B) as int8 and
# bitcasting the bytes as e5m2 IS exp(score*SCALE) to within ~3% (the
# mantissa-linear 2^f approximation) -- same order as the e5m2 grid the
# ScalarE path rounds onto anyway.  Offloading a fixed subset of exp
# groups to DVE relieves the ScalarE bottleneck (the only engine with a
# real exp) and keeps the PE the busiest engine, which also keeps HAM's
# activity monitor from clock-gating it.  B folds the e5m2 bias (15) and
# the half-max 2^f correction (-0.0436).
EXP_A = 4.0 * 1.4426950408889634 * SCALE
EXP_B = 4.0 * (15.0 - 0.0436)
# measured: DVE int8 stores run at HALF rate (2.1us per 1024-col call vs
# ScalarE's 1.11us) -- the offload only pays if DVE is otherwise idle,
# which it is not.  Disabled; ScalarE keeps all exp groups.
DVE_EXP_GROUPS = ()


def build_nc() -> bass.Bass:
    nc = bass.Bass()
    # hs and the q/k/v weights come in as fp8e4 in chunk-major layout
    # [128, chunk, n] so DoubleRow can pair feature chunks along the free
    # axis (the two k-tiles of a DR matmul must share partitions).
    hs2 = nc.declare_dram_parameter("hs2", [128, CCH * S], FP8, isOutput=False)
    res = nc.declare_dram_parameter("res", [C, SQ], F32, isOutput=False)
    wq2 = nc.declare_dram_parameter("wq2", [128, CCH * C], FP8, isOutput=False)
    wk2 = nc.declare_dram_parameter("wk2", [128, CCH * C], FP8, isOutput=False)
    wv2 = nc.declare_dram_parameter("wv2", [128, CCH * C], FP8, isOutput=False)
    woT = nc.declare_dram_parameter("woT", [C, C], BF16, isOutput=False)
    out = nc.declare_dram_parameter("out", [C, SQ], F32, isOutput=True)

    with ExitStack() as ctx:
        tc = ctx.enter_context(tile.TileContext(nc))
        # outer pool: tensors whose lifetime spans projections AND attention
        sb = ctx.enter_context(tc.tile_pool(name="sb", bufs=1))

        kT_sb = [sb.tile([128, S], BF16, tag=f"kT{i}", name=f"kT{i}") for i in range(CCH)]
        # per-head q, zero-padded to full 128-row contraction: partial-K
        # (K=64) matmuls drop HAM's utilization accounting below its 0.5
        # limit and clock the PE down to 1.2 GHz -- padding the contraction
        # with zero rows is exact and keeps the array at the warm rate.
        qTz_sb = [
            [sb.tile([128, SQ], BF16, tag=f"qz{i}_{p}", name=f"qz{i}_{p}")
             for p in range(2)]
            for i in range(CCH)
        ]
        # v: fp8, key-chunk pairs side by side (ktile stride 656 for DR)
        v_sb = [sb.tile([128, 2 * VKT], FP8, tag=f"v{j}", name=f"v{j}") for j in range(NJP)]
        # attention outputs packed per head pair (no zero padding)
        attn_sb = [sb.tile([128, SQ], BF16, tag=f"attn{h}", name=f"attn{h}")
                   for h in range(CCH)]
        ones_sb = sb.tile([128, DH], BF16, tag="ones", name="ones")
        nc.vector.memset(ones_sb[:], 1.0)

        # ---------------- load + first projections ----------------
        load = ctx.enter_context(tc.tile_pool(name="load", bufs=1))
        hs_sb = load.tile([128, CCH * S], FP8, tag="hs2", name="hs2")
        wk_sb = load.tile([128, CCH * C], FP8, tag="wk2", name="wk2")
        wq_sb = load.tile([128, CCH * C], FP8, tag="wq2", name="wq2")
        wv_sb = load.tile([128, CCH * C], FP8, tag="wv2", name="wv2")
        nc.sync.dma_start(wk_sb[:], wk2[:, :])
        for cc in range(CCH):
            nc.sync.dma_start(
                hs_sb[:, cc * S : cc * S + SQ], hs2[:, cc * S : cc * S + SQ]
            )
        nc.sync.dma_start(wq_sb[:], wq2[:, :])
        nc.sync.dma_start(wv_sb[:], wv2[:, :])
        h3 = hs_sb[:].rearrange("p (c s) -> p c s", c=CCH)
        k3 = wk_sb[:].rearrange("p (c f) -> p c f", c=CCH)
        q3 = wq_sb[:].rearrange("p (c f) -> p c f", c=CCH)
        v3w = wv_sb[:].rearrange("p (c f) -> p c f", c=CCH)

        def emit_hsT_tail():
            # deferred until after the first exp so ScalarE's conservative
            # vector-clock waits don't cover this 2MB of DMA
            for blk in range(SQ, S, SQ):
                for cc in range(CCH):
                    nc.sync.dma_start(
                        hs_sb[:, cc * S + blk : cc * S + blk + SQ],
                        hs2[:, cc * S + blk : cc * S + blk + SQ],
                    )

        def _proj_mm(ps, w3, wsl, hsl, dn=512):
            # contraction over 640 = 2 fp8 DoubleRow k-tile pairs + 1 plain
            nc.tensor.matmul(
                ps[:, 0:dn], w3[:, 0:2, wsl], h3[:, 0:2, hsl],
                start=True, stop=False,
                perf_mode=mybir.MatmulPerfMode.DoubleRow,
            )
            nc.tensor.matmul(
                ps[:, 0:dn], w3[:, 2:4, wsl], h3[:, 2:4, hsl],
                start=False, stop=False,
                perf_mode=mybir.MatmulPerfMode.DoubleRow,
            )
            nc.tensor.matmul(
                ps[:, 0:dn], w3[:, 4, wsl], h3[:, 4, hsl],
                start=False, stop=True,
            )

        def emit_kproj(dc, jt, pool):
            ps = pool.tile([128, 512], F32, tag="pp", name="pp", bufs=2)
            _proj_mm(ps, k3, ts(dc, 128), ts(jt, 512))
            nc.vector.tensor_copy(kT_sb[dc][:, ts(jt, 512)], ps[:])

        def emit_qproj(dc, it, pool):
            ps = pool.tile([128, 512], F32, tag="pp", name="pp", bufs=2)
            _proj_mm(ps, q3, ts(dc, 128), ts(it, 512))
            nc.vector.tensor_copy(qTz_sb[dc][0][0:DH, ts(it, 512)], ps[0:DH, :])
            nc.vector.tensor_copy(qTz_sb[dc][1][DH:128, ts(it, 512)], ps[DH:128, :])

        def emit_vproj(jc, pool):
            jp, half = divmod(jc, 2)
            vt = v_sb[jp]
            v3 = vt[:, half * VKT : half * VKT + H * VST].rearrange(
                "p (h x) -> p h x", x=VST
            )
            nc.vector.memset(v3[:, :, DH : DH + 1], 1.0)
            for d0, dn in ((0, 512), (512, 128)):
                ps = pool.tile([128, 512], F32, tag="pp", name="pp", bufs=2)
                # stationary = hs chunk pairs (M=128 keys), moving = wv
                nc.tensor.matmul(
                    ps[:, 0:dn], h3[:, 0:2, ts(jc, 128)],
                    v3w[:, 0:2, d0 : d0 + dn],
                    start=True, stop=False,
                    perf_mode=mybir.MatmulPerfMode.DoubleRow,
                )
                nc.tensor.matmul(
                    ps[:, 0:dn], h3[:, 2:4, ts(jc, 128)],
                    v3w[:, 2:4, d0 : d0 + dn],
                    start=False, stop=False,
                    perf_mode=mybir.MatmulPerfMode.DoubleRow,
                )
                nc.tensor.matmul(
                    ps[:, 0:dn], h3[:, 4, ts(jc, 128)],
                    v3w[:, 4, d0 : d0 + dn],
                    start=False, stop=True,
                )
                nc.vector.tensor_copy(
                    v3[:, d0 // DH : (d0 + dn) // DH, 0:DH],
                    ps[:, 0:dn].rearrange("p (h x) -> p h x", x=DH),
                )

        with tc.tile_pool(name="pp0", bufs=2, space="PSUM") as pp0:
            for dc in range(CCH):
                nc.vector.memset(qTz_sb[dc][0][DH:128, :], 0.0)
                nc.vector.memset(qTz_sb[dc][1][0:DH, :], 0.0)
            for jt in range(2):
                emit_kproj(0, jt, pp0)
            for it in range(NIT):
                emit_qproj(0, it, pp0)

        # ---------------- attention phase ----------------
        with tc.tile_pool(name="ap", bufs=1, space="PSUM") as ap, \
             tc.tile_pool(name="pt", bufs=6) as pt_pool, \
             tc.tile_pool(name="ob", bufs=3) as ob, \
             tc.tile_pool(name="scratch", bufs=4) as scratch:

            def norm_dve(hp, par, pv, p_isl):
                # copy numerator + denominator row out of PSUM (releases the
                # pv bank), then approx-reciprocal the denominators on DVE
                # (2.8x faster than InstReciprocal, ~2 ULP).
                raw = scratch.tile([DH, 512], BF16, tag="raw", name="raw")
                nc.vector.tensor_copy(raw[:], pv[0:DH, :])
                den = scratch.tile([DH + 1, 512], F32, tag="den", name="den")
                nc.vector.tensor_copy(den[DH : DH + 1, :], pv[DH : DH + 1, :])
                rec = scratch.tile([DH + 1, 512], BF16, tag="rec", name="rec")
                with nc.allow_low_precision(reason="softmax recip bf16"):
                    nc.vector.reciprocal(rec[DH : DH + 1, :], den[DH : DH + 1, :])
                return (hp, par, p_isl, raw, rec)

            def norm_mul(state, anchor=None):
                # rank-1 PE outer product broadcasts the reciprocal row
                # across partitions into a transient pp bank; DVE multiplies.
                # The tile scheduler orders by data deps, which would place
                # this right after the pair's last PV where it blocks the
                # strict-FIFO PE for the full ~3.4us reciprocal; pin it
                # behind a mid-next-pair QK instead.
                hp, par, p_isl, raw, rec = state
                rb = ap.tile([128, 512], F32, tag="pp", bufs=2, name="pp")
                r_mm = nc.tensor.matmul(
                    rb[0:DH, :],
                    ones_sb[DH : DH + 1, :],
                    rec[DH : DH + 1, :],
                    start=True,
                    stop=True,
                )
                if anchor is not None:
                    tile.add_dep_helper(
                        r_mm.ins, anchor.ins, sync=False,
                        reason="norm rank1 after anchor QK (hide recip latency)",
                    )
                nc.vector.tensor_mul(
                    attn_sb[hp][par * DH : (par + 1) * DH, p_isl],
                    raw[:],
                    rb[0:DH, :],
                )

            def emit_oproj(ec, it):
                wos = []
                for hp in range(CCH):
                    wt = ob.tile([128, 128], BF16, tag="woec", name="woec",
                                 bufs=25)
                    nc.sync.dma_start(wt[:], woT[ts(hp, 128), ts(ec, 128)])
                    wos.append(wt)
                ps = ap.tile([128, 512], F32, tag="pp", name="pp", bufs=2)
                for hp in range(CCH):
                    nc.tensor.matmul(
                        ps[:],
                        wos[hp][:],
                        attn_sb[hp][:, ts(it, 512)],
                        start=(hp == 0),
                        stop=(hp == CCH - 1),
                    )
                rt = ob.tile([128, 512], F32, tag="rt", name="rt", bufs=2)
                nc.sync.dma_start(rt[:], res[ts(ec, 128), ts(it, 512)])
                ot = ob.tile([128, 512], F32, tag="ot", name="ot", bufs=2)
                nc.vector.tensor_add(ot[:], ps[:], rt[:])
                nc.sync.dma_start(out[ts(ec, 128), ts(it, 512)], ot[:])

            pending = []
            # the last group's PVs of each pair are deferred into the next
            # pair, pinned behind its first QK: otherwise the chain
            # last-exp -> last-PV -> first-QK -> first-exp leaves ScalarE
            # idle ~3us at every pair boundary
            defer = [None]

            def flush_defer(anchor):
                d_hp, d_pvA, d_pvB, d_ptA, d_ptB, d_isl = defer[0]
                vv = v_sb[NJP - 1][:].rearrange("p (t z) -> p t z", t=2)
                for par, pv, pt in ((0, d_pvA, d_ptA), (1, d_pvB, d_ptB)):
                    h = 2 * d_hp + par
                    mm = nc.tensor.matmul(
                        pv[:],
                        vv[:, :, h * VST : h * VST + VST],
                        pt[:].rearrange("p (t n) -> p t n", t=2),
                        start=False,
                        stop=True,
                        perf_mode=mybir.MatmulPerfMode.DoubleRow,
                    )
                    if anchor is not None:
                        tile.add_dep_helper(
                            mm.ins, anchor.ins, sync=False,
                            reason="deferred last PV behind next pair's QK",
                        )
                pending.append(norm_dve(d_hp, 0, d_pvA, d_isl))
                pending.append(norm_dve(d_hp, 1, d_pvB, d_isl))
                defer[0] = None

            bg = [
                (lambda jt=jt: emit_kproj(0, jt, ap)) for jt in range(2, NJT)
            ]
            for it in range(NIT):
                isl = ts(it, 512)
                for hp in range(CCH):
                    if it == 0 and hp + 1 < CCH:
                        # prefetch next pair's K/Q chunks in the background,
                        # first-needed first: gi0 reads kT jt0 and it0's q,
                        # kT jt6/jt7 aren't touched until gi12/gi14, and it1's
                        # q not until the second query tile
                        bg.append(lambda dc=hp + 1: emit_kproj(dc, 0, ap))
                        bg.append(lambda dc=hp + 1: emit_qproj(dc, 0, ap))
                        bg.extend(
                            (lambda jt=jt, dc=hp + 1: emit_kproj(dc, jt, ap))
                            for jt in range(1, NJT)
                        )
                        bg.append(lambda dc=hp + 1: emit_qproj(dc, 1, ap))
                    if it == 1 and hp >= 1:
                        # spread the it0 output projections across the it1
                        # pairs: it1 has no projection prefetch, and a starved
                        # PE micro-idles every slot, making HAM oscillate
                        ecs = [hp - 1] if hp < 4 else [3, 4]
                        bg.extend(
                            (lambda ec=ec: emit_oproj(ec, 0)) for ec in ecs
                        )
                    pvA = ap.tile([DH + 1, 512], F32, tag="pv", bufs=2,
                                  name="pv")
                    pvB = ap.tile([DH + 1, 512], F32, tag="pv", bufs=2,
                                  name="pv")
                    for gi in range(NJP):
                        scA = ap.tile([128, 1024], F32, tag="scA", bufs=1,
                                      name="scA")
                        scB = ap.tile([128, 1024], F32, tag="scB", bufs=1,
                                      name="scB")
                        # full-K QK (zero-padded moving q keeps HAM warm)
                        last_qk = None
                        for k, jc in enumerate((2 * gi, 2 * gi + 1)):
                            nc.tensor.matmul(
                                scA[:, ts(k, 512)],
                                kT_sb[hp][:, ts(jc, 128)],
                                qTz_sb[hp][0][:, isl],
                                start=True,
                                stop=True,
                            )
                            last_qk = nc.tensor.matmul(
                                scB[:, ts(k, 512)],
                                kT_sb[hp][:, ts(jc, 128)],
                                qTz_sb[hp][1][:, isl],
                                start=True,
                                stop=True,
                            )
                        if gi == 0 and defer[0] is not None:
                            flush_defer(last_qk)
                        ptA = pt_pool.tile([128, 1024], PT8, tag="pt",
                                           name="pt")
                        ptB = pt_pool.tile([128, 1024], PT8, tag="pt",
                                           name="pt")
                        if gi in DVE_EXP_GROUPS:
                            for pt, sc in ((ptA, scA), (ptB, scB)):
                                nc.vector.tensor_scalar(
                                    pt[:].bitcast(mybir.dt.int8),
                                    sc[:],
                                    EXP_A,
                                    EXP_B,
                                    mybir.AluOpType.mult,
                                    mybir.AluOpType.add,
                                )
                        else:
                            for pt, sc in ((ptA, scA), (ptB, scB)):
                                nc.scalar.activation(
                                    pt[:], sc[:],
                                    mybir.ActivationFunctionType.Exp,
                                    bias=0.0, scale=SCALE,
                                )
                        # pop the previous pair's normalizations late enough
                        # that the ~3.4us DVE reciprocal has finished: the
                        # rank-1 broadcast sits in the strict-FIFO PE queue,
                        # and popping it early stalls every QK behind it
                        if pending and gi == 4:
                            norm_mul(pending.pop(0), last_qk)
                        if pending and gi == 6:
                            norm_mul(pending.pop(0), last_qk)
                        if it == 0 and hp == 0:
                            if gi == 0:
                                emit_hsT_tail()
                            # V projection rides inside the first pair's
                            # window, each chunk pair just ahead of its PV
                            if bg:
                                bg.pop(0)()
                            emit_vproj(2 * gi, ap)
                            emit_vproj(2 * gi + 1, ap)
                        elif bg:
                            bg.pop(0)()
                        # PV: fp8 DoubleRow over the key-chunk pair (the
                        # final group is deferred into the next pair)
                        if gi == NJP - 1:
                            defer[0] = (hp, pvA, pvB, ptA, ptB, isl)
                        else:
                            vv = v_sb[gi][:].rearrange("p (t z) -> p t z", t=2)
                            for par, pv, pt in ((0, pvA, ptA), (1, pvB, ptB)):
                                h = 2 * hp + par
                                nc.tensor.matmul(
                                    pv[:],
                                    vv[:, :, h * VST : h * VST + VST],
                                    pt[:].rearrange("p (t n) -> p t n", t=2),
                                    start=(gi == 0),
                                    stop=False,
                                    perf_mode=mybir.MatmulPerfMode.DoubleRow,
                                )
                    # barrier: next pair's kT/qT must be fully emitted
                    # before its first QK reads them
                    while bg:
                        bg.pop(0)()
            if defer[0] is not None:
                flush_defer(None)
            for st in pending:
                norm_mul(st)
            for ec in range(CCH):
                emit_oproj(ec, 1)

    _spill_matmul_waits(nc)
    return nc


# walrus embedded-sync-wait capacity per BIR opcode.  Matmult holds a
# single wait; excess waits hoist onto the paired Ldweights (in-order
# issue on PE makes that equivalent).  Other compute ops spill onto
# EventSemaphore carrier instructions inserted just before them on the
# same engine.  DMACopy / Drain / EventSemaphore handle many waits
# natively (bacc emits such itself) and are left alone.
_WAIT_CAPS = {
    "InstMatmult": 1,
    "InstLdweights": 1,
    "InstActivation": 1,
    "InstReciprocal": 1,
    "InstTensorTensor": 1,
    "InstTensorCopy": 1,
    "InstTensorScalarPtr": 1,
    "InstTensorReduce": 1,
    "InstMemset": 1,
    "InstDMACopy": 1,
    "InstDrain": 1,
    "InstCustomDveAnt": 1,
}
_ES_CAP = 2  # waits per EventSemaphore carrier (walrus: <=2 waits, <=1 update)


def _spill_matmul_waits(nc: bass.Bass) -> None:
    spill_id = [0]

    def carriers(excess, engine):
        out = []
        for i in range(0, len(excess), _ES_CAP):
            es = mybir.InstEventSemaphore(
                name=f"wait-spill-{spill_id[0]}", ins=[], outs=[]
            )
            spill_id[0] += 1
            es.engine = engine
            es.sync_info = mybir.SyncInfo(
                on_wait=excess[i : i + _ES_CAP], on_update=[]
            )
            out.append(es)
        return out

    for f in nc.m.functions:
        for blk in f.blocks:
            insts = blk.instructions
            i = 0
            while i < len(insts):
                inst = insts[i]
                tn = type(inst).__name__
                cap = _WAIT_CAPS.get(tn)
                si = inst.sync_info
                if cap is None or si is None or len(si.on_wait) <= cap:
                    i += 1
                    continue
                w = list(si.on_wait)
                if tn == "InstMatmult" and cap == 1:
                    # Keep the latest-satisfied dependency (the ACT-produced
                    # operand, e.g. probs from exp) embedded on the matmul and
                    # hoist early ones onto the Ldweights: a wait on the LDW
                    # blocks its background prefetch and serializes ~50ns of
                    # weight-load into every PV matmul.
                    acts = [x for x in w if "Activation" in (x.ant_name or "")]
                    if acts:
                        keep = [acts[-1]]
                        excess = [x for x in w if x is not acts[-1]]
                    else:
                        keep, excess = w[-cap:], w[:-cap]
                else:
                    keep, excess = w[-cap:], w[:-cap]
                prev = insts[i - 1] if i > 0 else None
                if (
                    tn == "InstMatmult"
                    and prev is not None
                    and type(prev).__name__ == "InstLdweights"
                    and len(((prev.sync_info and prev.sync_info.on_wait) or []))
                    + len(excess) <= 1
                ):
                    psi = prev.sync_info
                    pw = list(psi.on_wait) if psi is not None else []
                    pu = list(psi.on_update) if psi is not None else []
                    prev.sync_info = mybir.SyncInfo(on_wait=pw + excess, on_update=pu)
                else:
                    new = carriers(excess, inst.engine)
                    insts[i:i] = new
                    i += len(new)
                inst.sync_info = mybir.SyncInfo(
                    on_wait=keep, on_update=list(si.on_update)
                )
                i += 1


_CACHED_NC = None


def get_nc() -> bass.Bass:
    global _CACHED_NC
    if _CACHED_NC is None:
        _CACHED_NC = build_nc()
    return _CACHED_NC


def _chunk_major(mT, n, f8):
    # [C, n] -> [128, CCH*n] fp8: row p, col cc*n+j = mT[128*cc + p, j]
    return np.ascontiguousarray(
        mT.reshape(CCH, 128, n).transpose(1, 0, 2).reshape(128, CCH * n)
    ).astype(f8)


def make_in_maps(hidden_states, Wq, Wk, Wv, Wo, b_out):
    hs = np.asarray(hidden_states, dtype=np.float32)
    bf = ml_dtypes.bfloat16
    f8 = mybir.dt.np(FP8)
    wq2 = _chunk_major(np.asarray(Wq, np.float32).T, C, f8)
    wk2 = _chunk_major(np.asarray(Wk, np.float32).T, C, f8)
    wv2 = _chunk_major(np.asarray(Wv, np.float32).T, C, f8)
    woT = np.ascontiguousarray(np.asarray(Wo, np.float32).T).astype(bf)
    bias = np.asarray(b_out, np.float32).reshape(C, 1)
    in_maps = []
    for c in range(NCORES):
        b, g = divmod(c, GROUP)
        i0 = g * SQ
        hsTb = hs[b].T  # [C, S]
        in_maps.append(
            {
                "hs2": _chunk_major(np.roll(hsTb, -i0, axis=1), S, f8),
                "res": np.ascontiguousarray(hsTb[:, i0 : i0 + SQ]) + bias,
                "wq2": wq2,
                "wk2": wk2,
                "wv2": wv2,
                "woT": woT,
            }
        )
    return in_maps


def assemble(results) -> np.ndarray:
    y = np.empty((B, S, C), np.float32)
    for c in range(NCORES):
        b, g = divmod(c, GROUP)
        i0 = g * SQ
        y[b, i0 : i0 + SQ, :] = np.asarray(results[c]["out"], np.float32).T
    return y


def kernel(**inputs) -> np.ndarray:
    from concourse.bass_utils import run_bass_kernel_spmd

    nc = get_nc()
    in_maps = make_in_maps(**inputs)
    res = run_bass_kernel_spmd(nc, in_maps, list(range(NCORES)))
    return assemble(res.results)


if __name__ == "__main__":
    import reference

    inputs = {k: np.asarray(v) for k, v in reference.setup_inputs().items()}
    got = kernel(**inputs)
    want = np.asarray(reference.reference(**inputs))
    err = np.linalg.norm(got - want) / np.linalg.norm(want)
    print("Relative error:", err)


# revision 39
# speedup vs baseline: 1.0840x; 1.0043x over previous
"""Multi-head attention (AttnProcessor2_0) on 8 TRN2 NeuronCores.

Problem: B=2, S=4096, C=640, H=10, Dh=64.
  q/k/v = hs @ W{q,k,v}.T ; per-head scores = q k^T / 8 ; softmax ;
  out = probs v ; y = out @ Wo.T + b_out + hs

Sharding (no collectives): core c -> batch b=c//4, query block g=c%4
(1024 queries).  Each core recomputes full K/V for its batch (head-dim
on partitions), computes its own S/4 x S attention block, output
projection, bias+residual.  Host passes hidden states TRANSPOSED and
ROLLED by the query offset so the same SPMD program works on every
core (softmax+PV are permutation-invariant along the key axis).

v3 layout (vs the 514us baseline):
  * PV fp8 DoubleRow: probs written by ScalarE exp directly as fp8
    (e5m2 -- e4m3 stores cost ScalarE ~20% extra, e5m2 runs at bf16
    rate), v projected to fp8e4 with key-chunk PAIRS packed at stride
    656; each DR matmul contracts 256 keys (moving free dim 1024 >=
    256 so the ~1.5x DR win applies).  Denominators still fall out as
    PSUM row 64 via the ones column.  PV: 327,680 -> 163,840 cycles.
  * Q/K/V projections in fp8 DoubleRow: hs and Wq/Wk/Wv uploaded as
    fp8e4 in chunk-major [128, chunk, n] layout so DR pairs feature
    chunks along the free axis; 640-contraction = 2 DR + 1 plain
    matmul (5 -> 3 instructions per tile).
  * O proj de-padded: attention outputs are packed per head-PAIR
    [128, SQ], so the 640-contraction runs dense (25,600 cycles).
  * Normalization off the pv critical path: numerator+denominator rows
    are copied out of PSUM immediately (releasing the bank), the bf16
    rank-1 reciprocal broadcast goes to a transient pp bank, and the
    pending broadcasts pop at gi 5/7 of the NEXT pair so the ~3.4us
    DVE reciprocal never stalls the strict-FIFO PE queue.
  * QK stays zero-padded full-K (K=64 matmuls or 64-row tiles drop
    HAM's utilization accounting below its limit and the PE is
    clock-gated to 1.2 GHz).  PSUM: 4 banks scores (A/B ping-pong) +
    2 pv + 2 background-projection banks.
"""

import sys

if "/opt/trn_rl_repo" not in sys.path:
    sys.path.insert(0, "/opt/trn_rl_repo")

from contextlib import ExitStack

import ml_dtypes
import numpy as np

import concourse.bass as bass
import concourse.tile as tile
from concourse import mybir
from concourse.bass import ts

BF16 = mybir.dt.bfloat16
F32 = mybir.dt.float32
FP8 = mybir.dt.float8e4  # v tiles (e4m3: 3-bit mantissa)
PT8 = mybir.dt.float8e5  # probs from exp (e5m2)

B, S, C = 2, 4096, 640
H, DH = 10, 64
NCORES = 8
GROUP = 4  # cores per batch element
SQ = S // GROUP  # 1024 queries per core
SCALE = 0.125  # 1/sqrt(64)
CCH = C // 128  # 5 feature chunks (2 heads each)
NJT = S // 512  # 8 key tiles for K proj
NJC = S // 128  # 32 key chunks for attention
NJP = NJC // 2  # 16 key chunk PAIRS (DoubleRow k-tiles)
NIT = SQ // 512  # 2 query tiles
VST = DH + 1  # 65: per-head stride in v tiles (ones col at f=64)
VKT = 656  # per-ktile stride in v pair tiles (10*65=650 padded to %16==0)

# Schraudolph exp-as-int8 on DVE: writing round(A*score + B) as int8 and
# bitcasting the bytes as e5m2 IS exp(score*SCALE) to within ~3% (the
# mantissa-linear 2^f approximation) -- same order as the e5m2 grid the
# ScalarE path rounds onto anyway.  Offloading a fixed subset of exp
# groups to DVE relieves the ScalarE bottleneck (the only engine with a
# real exp) and keeps the PE the busiest engine, which also keeps HAM's
# activity monitor from clock-gating it.  B folds the e5m2 bias (15) and
# the half-max 2^f correction (-0.0436).
EXP_A = 4.0 * 1.4426950408889634 * SCALE
EXP_B = 4.0 * (15.0 - 0.0436)
# measured: DVE int8 stores run at HALF rate (2.1us per 1024-col call vs
# ScalarE's 1.11us) -- the offload only pays if DVE is otherwise idle,
# which it is not.  Disabled; ScalarE keeps all exp groups.
DVE_EXP_GROUPS = ()


def build_nc() -> bass.Bass:
    nc = bass.Bass()
    # hs and the q/k/v weights come in as fp8e4 in chunk-major layout
    # [128, chunk, n] so DoubleRow can pair feature chunks along the free
    # axis (the two k-tiles of a DR matmul must share partitions).
    hs2 = nc.declare_dram_parameter("hs2", [128, CCH * S], FP8, isOutput=False)
    res = nc.declare_dram_parameter("res", [C, SQ], F32, isOutput=False)
    wq2 = nc.declare_dram_parameter("wq2", [128, CCH * C], FP8, isOutput=False)
    wk2 = nc.declare_dram_parameter("wk2", [128, CCH * C], FP8, isOutput=False)
    wv2 = nc.declare_dram_parameter("wv2", [128, CCH * C], FP8, isOutput=False)
    woT = nc.declare_dram_parameter("woT", [C, C], BF16, isOutput=False)
    out = nc.declare_dram_parameter("out", [C, SQ], F32, isOutput=True)

    with ExitStack() as ctx:
        tc = ctx.enter_context(tile.TileContext(nc))
        # outer pool: tensors whose lifetime spans projections AND attention
        sb = ctx.enter_context(tc.tile_pool(name="sb", bufs=1))

        kT_sb = [sb.tile([128, S], BF16, tag=f"kT{i}", name=f"kT{i}") for i in range(CCH)]
        # per-head q, zero-padded to full 128-row contraction: partial-K
        # (K=64) matmuls drop HAM's utilization accounting below its 0.5
        # limit and clock the PE down to 1.2 GHz -- padding the contraction
        # with zero rows is exact and keeps the array at the warm rate.
        qTz_sb = [
            [sb.tile([128, SQ], BF16, tag=f"qz{i}_{p}", name=f"qz{i}_{p}")
             for p in range(2)]
            for i in range(CCH)
        ]
        # v: fp8, key-chunk pairs side by side (ktile stride 656 for DR)
        v_sb = [sb.tile([128, 2 * VKT], FP8, tag=f"v{j}", name=f"v{j}") for j in range(NJP)]
        # attention outputs packed per head pair (no zero padding)
        attn_sb = [sb.tile([128, SQ], BF16, tag=f"attn{h}", name=f"attn{h}")
                   for h in range(CCH)]
        ones_sb = sb.tile([128, DH], BF16, tag="ones", name="ones")
        nc.vector.memset(ones_sb[:], 1.0)

        # ---------------- load + first projections ----------------
        load = ctx.enter_context(tc.tile_pool(name="load", bufs=1))
        hs_sb = load.tile([128, CCH * S], FP8, tag="hs2", name="hs2")
        wk_sb = load.tile([128, CCH * C], FP8, tag="wk2", name="wk2")
        wq_sb = load.tile([128, CCH * C], FP8, tag="wq2", name="wq2")
        wv_sb = load.tile([128, CCH * C], FP8, tag="wv2", name="wv2")
        nc.sync.dma_start(wk_sb[:], wk2[:, :])
        for cc in range(CCH):
            nc.sync.dma_start(
                hs_sb[:, cc * S : cc * S + SQ], hs2[:, cc * S : cc * S + SQ]
            )
        nc.sync.dma_start(wq_sb[:], wq2[:, :])
        nc.sync.dma_start(wv_sb[:], wv2[:, :])
        h3 = hs_sb[:].rearrange("p (c s) -> p c s", c=CCH)
        k3 = wk_sb[:].rearrange("p (c f) -> p c f", c=CCH)
        q3 = wq_sb[:].rearrange("p (c f) -> p c f", c=CCH)
        v3w = wv_sb[:].rearrange("p (c f) -> p c f", c=CCH)

        def emit_hsT_tail():
            # deferred until after the first exp so ScalarE's conservative
            # vector-clock waits don't cover this 2MB of DMA
            for blk in range(SQ, S, SQ):
                for cc in range(CCH):
                    nc.sync.dma_start(
                        hs_sb[:, cc * S + blk : cc * S + blk + SQ],
                        hs2[:, cc * S + blk : cc * S + blk + SQ],
                    )

        def _proj_mm(ps, w3, wsl, hsl, dn=512):
            # contraction over 640 = 2 fp8 DoubleRow k-tile pairs + 1 plain
            nc.tensor.matmul(
                ps[:, 0:dn], w3[:, 0:2, wsl], h3[:, 0:2, hsl],
                start=True, stop=False,
                perf_mode=mybir.MatmulPerfMode.DoubleRow,
            )
            nc.tensor.matmul(
                ps[:, 0:dn], w3[:, 2:4, wsl], h3[:, 2:4, hsl],
                start=False, stop=False,
                perf_mode=mybir.MatmulPerfMode.DoubleRow,
            )
            nc.tensor.matmul(
                ps[:, 0:dn], w3[:, 4, wsl], h3[:, 4, hsl],
                start=False, stop=True,
            )

        def emit_kproj(dc, jt, pool):
            ps = pool.tile([128, 512], F32, tag="pp", name="pp", bufs=2)
            _proj_mm(ps, k3, ts(dc, 128), ts(jt, 512))
            nc.vector.tensor_copy(kT_sb[dc][:, ts(jt, 512)], ps[:])

        def emit_qproj(dc, it, pool):
            ps = pool.tile([128, 512], F32, tag="pp", name="pp", bufs=2)
            _proj_mm(ps, q3, ts(dc, 128), ts(it, 512))
            nc.vector.tensor_copy(qTz_sb[dc][0][0:DH, ts(it, 512)], ps[0:DH, :])
            nc.vector.tensor_copy(qTz_sb[dc][1][DH:128, ts(it, 512)], ps[DH:128, :])

        def emit_vproj(jc, pool):
            jp, half = divmod(jc, 2)
            vt = v_sb[jp]
            v3 = vt[:, half * VKT : half * VKT + H * VST].rearrange(
                "p (h x) -> p h x", x=VST
            )
            nc.vector.memset(v3[:, :, DH : DH + 1], 1.0)
            for d0, dn in ((0, 512), (512, 128)):
                ps = pool.tile([128, 512], F32, tag="pp", name="pp", bufs=2)
                # stationary = hs chunk pairs (M=128 keys), moving = wv
                nc.tensor.matmul(
                    ps[:, 0:dn], h3[:, 0:2, ts(jc, 128)],
                    v3w[:, 0:2, d0 : d0 + dn],
                    start=True, stop=False,
                    perf_mode=mybir.MatmulPerfMode.DoubleRow,
                )
                nc.tensor.matmul(
                    ps[:, 0:dn], h3[:, 2:4, ts(jc, 128)],
                    v3w[:, 2:4, d0 : d0 + dn],
                    start=False, stop=False,
                    perf_mode=mybir.MatmulPerfMode.DoubleRow,
                )
                nc.tensor.matmul(
                    ps[:, 0:dn], h3[:, 4, ts(jc, 128)],
                    v3w[:, 4, d0 : d0 + dn],
                    start=False, stop=True,
                )
                nc.vector.tensor_copy(
                    v3[:, d0 // DH : (d0 + dn) // DH, 0:DH],
                    ps[:, 0:dn].rearrange("p (h x) -> p h x", x=DH),
                )

        with tc.tile_pool(name="pp0", bufs=2, space="PSUM") as pp0:
            # only dc0's zero-padding gates the first QK; the other four
            # pairs' memsets would otherwise queue ~9us of DVE ahead of the
            # first q-projection copy on the startup critical path
            nc.vector.memset(qTz_sb[0][0][DH:128, :], 0.0)
            nc.vector.memset(qTz_sb[0][1][0:DH, :], 0.0)
            for jt in range(2):
                emit_kproj(0, jt, pp0)
            for it in range(NIT):
                emit_qproj(0, it, pp0)
            for dc in range(1, CCH):
                nc.vector.memset(qTz_sb[dc][0][DH:128, :], 0.0)
                nc.vector.memset(qTz_sb[dc][1][0:DH, :], 0.0)

        # ---------------- attention phase ----------------
        with tc.tile_pool(name="ap", bufs=1, space="PSUM") as ap, \
             tc.tile_pool(name="pt", bufs=6) as pt_pool, \
             tc.tile_pool(name="ob", bufs=3) as ob, \
             tc.tile_pool(name="scratch", bufs=4) as scratch:

            def norm_dve(hp, par, pv, p_isl):
                # copy numerator + denominator row out of PSUM (releases the
                # pv bank), then approx-reciprocal the denominators on DVE
                # (2.8x faster than InstReciprocal, ~2 ULP).
                raw = scratch.tile([DH, 512], BF16, tag="raw", name="raw")
                nc.vector.tensor_copy(raw[:], pv[0:DH, :])
                den = scratch.tile([DH + 1, 512], F32, tag="den", name="den")
                nc.vector.tensor_copy(den[DH : DH + 1, :], pv[DH : DH + 1, :])
                rec = scratch.tile([DH + 1, 512], BF16, tag="rec", name="rec")
                with nc.allow_low_precision(reason="softmax recip bf16"):
                    nc.vector.reciprocal(rec[DH : DH + 1, :], den[DH : DH + 1, :])
                return (hp, par, p_isl, raw, rec)

            def norm_mul(state, anchor=None):
                # rank-1 PE outer product broadcasts the reciprocal row
                # across partitions into a transient pp bank; DVE multiplies.
                # The tile scheduler orders by data deps, which would place
                # this right after the pair's last PV where it blocks the
                # strict-FIFO PE for the full ~3.4us reciprocal; pin it
                # behind a mid-next-pair QK instead.
                hp, par, p_isl, raw, rec = state
                rb = ap.tile([128, 512], F32, tag="pp", bufs=2, name="pp")
                r_mm = nc.tensor.matmul(
                    rb[0:DH, :],
                    ones_sb[DH : DH + 1, :],
                    rec[DH : DH + 1, :],
                    start=True,
                    stop=True,
                )
                if anchor is not None:
                    tile.add_dep_helper(
                        r_mm.ins, anchor.ins, sync=False,
                        reason="norm rank1 after anchor QK (hide recip latency)",
                    )
                nc.vector.tensor_mul(
                    attn_sb[hp][par * DH : (par + 1) * DH, p_isl],
                    raw[:],
                    rb[0:DH, :],
                )

            def emit_oproj(ec, it):
                wos = []
                for hp in range(CCH):
                    wt = ob.tile([128, 128], BF16, tag="woec", name="woec",
                                 bufs=25)
                    nc.sync.dma_start(wt[:], woT[ts(hp, 128), ts(ec, 128)])
                    wos.append(wt)
                ps = ap.tile([128, 512], F32, tag="pp", name="pp", bufs=2)
                for hp in range(CCH):
                    nc.tensor.matmul(
                        ps[:],
                        wos[hp][:],
                        attn_sb[hp][:, ts(it, 512)],
                        start=(hp == 0),
                        stop=(hp == CCH - 1),
                    )
                rt = ob.tile([128, 512], F32, tag="rt", name="rt", bufs=2)
                nc.sync.dma_start(rt[:], res[ts(ec, 128), ts(it, 512)])
                ot = ob.tile([128, 512], F32, tag="ot", name="ot", bufs=2)
                nc.vector.tensor_add(ot[:], ps[:], rt[:])
                nc.sync.dma_start(out[ts(ec, 128), ts(it, 512)], ot[:])

            pending = []
            # the last group's PVs of each pair are deferred into the next
            # pair, pinned behind its first QK: otherwise the chain
            # last-exp -> last-PV -> first-QK -> first-exp leaves ScalarE
            # idle ~3us at every pair boundary
            defer = [None]

            def flush_defer(anchor):
                d_hp, d_pvA, d_pvB, d_ptA, d_ptB, d_isl = defer[0]
                vv = v_sb[NJP - 1][:].rearrange("p (t z) -> p t z", t=2)
                for par, pv, pt in ((0, d_pvA, d_ptA), (1, d_pvB, d_ptB)):
                    h = 2 * d_hp + par
                    mm = nc.tensor.matmul(
                        pv[:],
                        vv[:, :, h * VST : h * VST + VST],
                        pt[:].rearrange("p (t n) -> p t n", t=2),
                        start=False,
                        stop=True,
                        perf_mode=mybir.MatmulPerfMode.DoubleRow,
                    )
                    if anchor is not None:
                        tile.add_dep_helper(
                            mm.ins, anchor.ins, sync=False,
                            reason="deferred last PV behind next pair's QK",
                        )
                pending.append(norm_dve(d_hp, 0, d_pvA, d_isl))
                pending.append(norm_dve(d_hp, 1, d_pvB, d_isl))
                defer[0] = None

            bg = [
                (lambda jt=jt: emit_kproj(0, jt, ap)) for jt in range(2, NJT)
            ]
            for it in range(NIT):
                isl = ts(it, 512)
                for hp in range(CCH):
                    if it == 0 and hp + 1 < CCH:
                        # prefetch next pair's K/Q chunks in the background,
                        # first-needed first: gi0 reads kT jt0 and it0's q,
                        # kT jt6/jt7 aren't touched until gi12/gi14, and it1's
                        # q not until the second query tile
                        bg.append(lambda dc=hp + 1: emit_kproj(dc, 0, ap))
                        bg.append(lambda dc=hp + 1: emit_qproj(dc, 0, ap))
                        bg.extend(
                            (lambda jt=jt, dc=hp + 1: emit_kproj(dc, jt, ap))
                            for jt in range(1, NJT)
                        )
                        bg.append(lambda dc=hp + 1: emit_qproj(dc, 1, ap))
                    if it == 1 and hp >= 1:
                        # spread the it0 output projections across the it1
                        # pairs: it1 has no projection prefetch, and a starved
                        # PE micro-idles every slot, making HAM oscillate
                        ecs = [hp - 1] if hp < 4 else [3, 4]
                        bg.extend(
                            (lambda ec=ec: emit_oproj(ec, 0)) for ec in ecs
                        )
                    pvA = ap.tile([DH + 1, 512], F32, tag="pv", bufs=2,
                                  name="pv")
                    pvB = ap.tile([DH + 1, 512], F32, tag="pv", bufs=2,
                                  name="pv")
                    for gi in range(NJP):
                        scA = ap.tile([128, 1024], F32, tag="scA", bufs=1,
                                      name="scA")
                        scB = ap.tile([128, 1024], F32, tag="scB", bufs=1,
                                      name="scB")
                        # full-K QK (zero-padded moving q keeps HAM warm)
                        last_qk = None
                        for k, jc in enumerate((2 * gi, 2 * gi + 1)):
                            nc.tensor.matmul(
                                scA[:, ts(k, 512)],
                                kT_sb[hp][:, ts(jc, 128)],
                                qTz_sb[hp][0][:, isl],
                                start=True,
                                stop=True,
                            )
                            last_qk = nc.tensor.matmul(
                                scB[:, ts(k, 512)],
                                kT_sb[hp][:, ts(jc, 128)],
                                qTz_sb[hp][1][:, isl],
                                start=True,
                                stop=True,
                            )
                        if gi == 0 and defer[0] is not None:
                            flush_defer(last_qk)
                        ptA = pt_pool.tile([128, 1024], PT8, tag="pt",
                                           name="pt")
                        ptB = pt_pool.tile([128, 1024], PT8, tag="pt",
                                           name="pt")
                        if gi in DVE_EXP_GROUPS:
                            for pt, sc in ((ptA, scA), (ptB, scB)):
                                nc.vector.tensor_scalar(
                                    pt[:].bitcast(mybir.dt.int8),
                                    sc[:],
                                    EXP_A,
                                    EXP_B,
                                    mybir.AluOpType.mult,
                                    mybir.AluOpType.add,
                                )
                        else:
                            for pt, sc in ((ptA, scA), (ptB, scB)):
                                nc.scalar.activation(
                                    pt[:], sc[:],
                                    mybir.ActivationFunctionType.Exp,
                                    bias=0.0, scale=SCALE,
                                )
                        # pop the previous pair's normalizations late enough
                        # that the ~3.4us DVE reciprocal has finished: the
                        # rank-1 broadcast sits in the strict-FIFO PE queue,
                        # and popping it early stalls every QK behind it
                        if pending and gi == 4:
                            norm_mul(pending.pop(0), last_qk)
                        if pending and gi == 6:
                            norm_mul(pending.pop(0), last_qk)
                        if it == 0 and hp == 0:
                            if gi == 0:
                                emit_hsT_tail()
                            # V projection rides inside the first pair's
                            # window, each chunk pair just ahead of its PV
                            if bg:
                                bg.pop(0)()
                            emit_vproj(2 * gi, ap)
                            emit_vproj(2 * gi + 1, ap)
                        elif bg:
                            bg.pop(0)()
                        # PV: fp8 DoubleRow over the key-chunk pair (the
                        # final group is deferred into the next pair)
                        if gi == NJP - 1:
                            defer[0] = (hp, pvA, pvB, ptA, ptB, isl)
                        else:
                            vv = v_sb[gi][:].rearrange("p (t z) -> p t z", t=2)
                            for par, pv, pt in ((0, pvA, ptA), (1, pvB, ptB)):
                                h = 2 * hp + par
                                nc.tensor.matmul(
                                    pv[:],
                                    vv[:, :, h * VST : h * VST + VST],
                                    pt[:].rearrange("p (t n) -> p t n", t=2),
                                    start=(gi == 0),
                                    stop=False,
                                    perf_mode=mybir.MatmulPerfMode.DoubleRow,
                                )
                    # barrier: next pair's kT/qT must be fully emitted
                    # before its first QK reads them
                    while bg:
                        bg.pop(0)()
            if defer[0] is not None:
                flush_defer(None)
            for st in pending:
                norm_mul(st)
            for ec in range(CCH):
                emit_oproj(ec, 1)

    _spill_matmul_waits(nc)
    return nc


# walrus embedded-sync-wait capacity per BIR opcode.  Matmult holds a
# single wait; excess waits hoist onto the paired Ldweights (in-order
# issue on PE makes that equivalent).  Other compute ops spill onto
# EventSemaphore carrier instructions inserted just before them on the
# same engine.  DMACopy / Drain / EventSemaphore handle many waits
# natively (bacc emits such itself) and are left alone.
_WAIT_CAPS = {
    "InstMatmult": 1,
    "InstLdweights": 1,
    "InstActivation": 1,
    "InstReciprocal": 1,
    "InstTensorTensor": 1,
    "InstTensorCopy": 1,
    "InstTensorScalarPtr": 1,
    "InstTensorReduce": 1,
    "InstMemset": 1,
    "InstDMACopy": 1,
    "InstDrain": 1,
    "InstCustomDveAnt": 1,
}
_ES_CAP = 2  # waits per EventSemaphore carrier (walrus: <=2 waits, <=1 update)


def _spill_matmul_waits(nc: bass.Bass) -> None:
    spill_id = [0]

    def carriers(excess, engine):
        out = []
        for i in range(0, len(excess), _ES_CAP):
            es = mybir.InstEventSemaphore(
                name=f"wait-spill-{spill_id[0]}", ins=[], outs=[]
            )
            spill_id[0] += 1
            es.engine = engine
            es.sync_info = mybir.SyncInfo(
                on_wait=excess[i : i + _ES_CAP], on_update=[]
            )
            out.append(es)
        return out

    for f in nc.m.functions:
        for blk in f.blocks:
            insts = blk.instructions
            i = 0
            while i < len(insts):
                inst = insts[i]
                tn = type(inst).__name__
                cap = _WAIT_CAPS.get(tn)
                si = inst.sync_info
                if cap is None or si is None or len(si.on_wait) <= cap:
                    i += 1
                    continue
                w = list(si.on_wait)
                if tn == "InstMatmult" and cap == 1:
                    # Keep the latest-satisfied dependency (the ACT-produced
                    # operand, e.g. probs from exp) embedded on the matmul and
                    # hoist early ones onto the Ldweights: a wait on the LDW
                    # blocks its background prefetch and serializes ~50ns of
                    # weight-load into every PV matmul.
                    acts = [x for x in w if "Activation" in (x.ant_name or "")]
                    if acts:
                        keep = [acts[-1]]
                        excess = [x for x in w if x is not acts[-1]]
                    else:
                        keep, excess = w[-cap:], w[:-cap]
                else:
                    keep, excess = w[-cap:], w[:-cap]
                prev = insts[i - 1] if i > 0 else None
                if (
                    tn == "InstMatmult"
                    and prev is not None
                    and type(prev).__name__ == "InstLdweights"
                    and len(((prev.sync_info and prev.sync_info.on_wait) or []))
                    + len(excess) <= 1
                ):
                    psi = prev.sync_info
                    pw = list(psi.on_wait) if psi is not None else []
                    pu = list(psi.on_update) if psi is not None else []
                    prev.sync_info = mybir.SyncInfo(on_wait=pw + excess, on_update=pu)
                else:
                    new = carriers(excess, inst.engine)
                    insts[i:i] = new
                    i += len(new)
                inst.sync_info = mybir.SyncInfo(
                    on_wait=keep, on_update=list(si.on_update)
                )
                i += 1


_CACHED_NC = None


def get_nc() -> bass.Bass:
    global _CACHED_NC
    if _CACHED_NC is None:
        _CACHED_NC = build_nc()
    return _CACHED_NC


def _chunk_major(mT, n, f8):
    # [C, n] -> [128, CCH*n] fp8: row p, col cc*n+j = mT[128*cc + p, j]
    return np.ascontiguousarray(
        mT.reshape(CCH, 128, n).transpose(1, 0, 2).reshape(128, CCH * n)
    ).astype(f8)


def make_in_maps(hidden_states, Wq, Wk, Wv, Wo, b_out):
    hs = np.asarray(hidden_states, dtype=np.float32)
    bf = ml_dtypes.bfloat16
    f8 = mybir.dt.np(FP8)
    wq2 = _chunk_major(np.asarray(Wq, np.float32).T, C, f8)
    wk2 = _chunk_major(np.asarray(Wk, np.float32).T, C, f8)
    wv2 = _chunk_major(np.asarray(Wv, np.float32).T, C, f8)
    woT = np.ascontiguousarray(np.asarray(Wo, np.float32).T).astype(bf)
    bias = np.asarray(b_out, np.float32).reshape(C, 1)
    in_maps = []
    for c in range(NCORES):
        b, g = divmod(c, GROUP)
        i0 = g * SQ
        hsTb = hs[b].T  # [C, S]
        in_maps.append(
            {
                "hs2": _chunk_major(np.roll(hsTb, -i0, axis=1), S, f8),
                "res": np.ascontiguousarray(hsTb[:, i0 : i0 + SQ]) + bias,
                "wq2": wq2,
                "wk2": wk2,
                "wv2": wv2,
                "woT": woT,
            }
        )
    return in_maps


def assemble(results) -> np.ndarray:
    y = np.empty((B, S, C), np.float32)
    for c in range(NCORES):
        b, g = divmod(c, GROUP)
        i0 = g * SQ
        y[b, i0 : i0 + SQ, :] = np.asarray(results[c]["out"], np.float32).T
    return y


def kernel(**inputs) -> np.ndarray:
    from concourse.bass_utils import run_bass_kernel_spmd

    nc = get_nc()
    in_maps = make_in_maps(**inputs)
    res = run_bass_kernel_spmd(nc, in_maps, list(range(NCORES)))
    return assemble(res.results)


if __name__ == "__main__":
    import reference

    inputs = {k: np.asarray(v) for k, v in reference.setup_inputs().items()}
    got = kernel(**inputs)
    want = np.asarray(reference.reference(**inputs))
    err = np.linalg.norm(got - want) / np.linalg.norm(want)
    print("Relative error:", err)
